# revision 61
# baseline (speedup 1.0000x reference)
"""Two-layer GAT (GATConv x2 + log_softmax) on 8 Trainium2 NeuronCores.

Strategy (edge-parallel by dst node, zero on-device gathers):
  Host does pure index relabeling (argsort / np.take / dtype casts); all
  FLOPs run on-device across 3 SPMD launches.

  L0: nodes-on-partitions. One bf16 matmul per 128-node tile against
      Wcat = [W1 | W1@a_src | W1@a_dst] (built on device) yields h1 and
      both attention scores s_src/s_dst in a single pass.
  host: edges sorted by dst; per-node degree-bucketed padded slot layout
      (granularity 8), buckets dealt round-robin across the 8 cores so
      every core gets an identical block layout; per-slot streams:
      s_src (1 bf16) + h1 rows (8 bf16), dummy slots get s_src = -100
      so their softmax weight underflows to ~0.
  L1: e = s_src + s_dst (DVE/Pool), w = Exp(Prelu(e, 0.2)) on the ACT
      engine written straight into a 9th feature row of the q tile, so
      the z = sum(w) rides the same halving add-tree as the weighted
      feature sums. mults/trees split across DVE+GPSIMD by a per-chunk
      schedule; out1 = agg/(z+1e-16)+b1, relu, h2 = @W2, s2 scores.
  host: same relabeling for layer-2 streams (h2 + s2 scores).
  L2: same reduction with 2 features + w row, then log_softmax (f32).
"""

import numpy as np
import ml_dtypes

import concourse.bass as bass
import concourse.bacc as bacc
import concourse.mybir as mybir
import concourse.tile as tile
from concourse.bass_utils import run_bass_kernel_spmd

bf16 = ml_dtypes.bfloat16
P = 128
N_CORES = 8
GRAN = 8
SENTINEL = -100.0
F32 = mybir.dt.float32
BF16 = mybir.dt.bfloat16
AF = mybir.ActivationFunctionType
ALU = mybir.AluOpType
AX = mybir.AxisListType


# ----------------------------------------------------------------------
# Host-side plan (index arithmetic only)
# ----------------------------------------------------------------------

def build_plan(dst, n_nodes):
    E = dst.shape[0]
    order = np.argsort(dst, kind="stable").astype(np.int64)
    deg = np.bincount(dst, minlength=n_nodes).astype(np.int64)
    starts = np.zeros(n_nodes + 1, np.int64)
    np.cumsum(deg, out=starts[1:])

    Dpad = np.maximum(((deg + GRAN - 1) // GRAN) * GRAN, GRAN)
    Ds = np.unique(Dpad)

    # deal each bucket's nodes round-robin across cores: per-core counts are
    # equal (+-1) so the unified block counts carry no cross-core padding
    nblk = {}
    percore_nodes = [dict() for _ in range(N_CORES)]
    for D in Ds:
        sel = np.nonzero(Dpad == D)[0]
        for c in range(N_CORES):
            mine = sel[c::N_CORES]
            percore_nodes[c][int(D)] = mine
            nb = (len(mine) + P - 1) // P
            nblk[int(D)] = max(nblk.get(int(D), 0), nb)
    buckets = [(int(D), nblk[int(D)]) for D in Ds if nblk[int(D)] > 0]
    bounds = None

    n_pb = sum(nb for _, nb in buckets)             # node-blocks per partition
    S_part = sum(D * nb for D, nb in buckets)       # slots per partition

    cores = []
    for c in range(N_CORES):
        node_order = []
        for D, nb in buckets:
            sel = percore_nodes[c][D]
            padded = np.full(nb * P, -1, np.int64)
            padded[: len(sel)] = sel
            node_order.append(padded)
        node_order = np.concatenate(node_order)      # [(bucket,block,partition)]
        # perm: [P, S_part] edge index or -1, laid out per partition as
        # concat over buckets of [nblk, D]
        perm = np.full((P, S_part), -1, np.int64)
        no = node_order.reshape(n_pb, P)
        off = 0
        bi = 0
        for D, nb in buckets:
            for j in range(nb):
                nid = no[bi + j]                      # [P]
                for p in range(P):
                    n = nid[p]
                    if n < 0:
                        continue
                    s0 = starts[n]
                    cdeg = deg[n]
                    perm[p, off + j * D : off + j * D + cdeg] = order[s0 : s0 + cdeg]
            off += nb * D
            bi += nb
        cores.append({"node_order": node_order, "perm": perm})

    return {
        "buckets": buckets,
        "n_pb": n_pb,
        "S_part": S_part,
        "bounds": bounds,
        "cores": cores,
    }


def build_streams(plan, core_idx, src, s_tab, H_tab):
    """s_tab [N] f32 (src attention score), H_tab [N,F] bf16. Returns
    SA bf16 [P, S_part] flat s_src per slot (dummy slots = SENTINEL so
    their softmax weight underflows to ~0) and SH bf16
    [P, sum F*D*nblk] with per-bucket layout [nblk, F, D] per part."""
    core = plan["cores"][core_idx]
    perm = core["perm"]
    valid = perm >= 0
    Fh = H_tab.shape[1]
    src_slot = np.where(valid, src[np.clip(perm, 0, None)], 0)
    Aslot = s_tab[src_slot].astype(np.float32)   # [P, S]
    Aslot[~valid] = SENTINEL
    Hslot = H_tab[src_slot]                      # [P, S, F]
    Hslot[~valid] = 0
    parts = []
    off = 0
    for D, nb in plan["buckets"]:
        n = nb * D
        h = Hslot[:, off : off + n].reshape(P, nb, D, Fh).transpose(0, 1, 3, 2)
        parts.append(np.ascontiguousarray(h).reshape(P, -1).astype(bf16))
        off += n
    return Aslot.astype(bf16), np.concatenate(parts, axis=1)


def build_tbe(plan, core_idx, tab):
    """tab [N] f32 -> [P, S_part] bf16: per-slot s_dst (node value repeated
    across its D slots; zeros for dummy nodes)."""
    core = plan["cores"][core_idx]
    no = core["node_order"].reshape(plan["n_pb"], P)
    valid = no >= 0
    t = tab[np.clip(no, 0, None)].copy()         # [n_pb, P]
    t[~valid] = 0.0
    parts = []
    nbo = 0
    for D, nb in plan["buckets"]:
        blk = t[nbo : nbo + nb].T                # [P, nb]
        parts.append(np.repeat(blk[:, :, None], D, axis=2).reshape(P, nb * D))
        nbo += nb
    return np.concatenate(parts, axis=1).astype(bf16)


def build_node_tab(plan, core_idx, tab, k):
    """tab [N,k] f32 -> [P, n_pb, k] per layout order (zeros for dummies)."""
    core = plan["cores"][core_idx]
    no = core["node_order"].reshape(plan["n_pb"], P)
    valid = no >= 0
    t = tab[np.clip(no, 0, None)].copy()         # [n_pb, P, k]
    t[~valid] = 0.0
    return np.ascontiguousarray(t.transpose(1, 0, 2)).astype(bf16)


# ----------------------------------------------------------------------
# Launch 0: h1 = x@W1, s_src/s_dst scores - nodes on partitions
# ----------------------------------------------------------------------
NC_NODES = 12500
NC_PAD = 12544          # 98 tiles of 128
NT0 = NC_PAD // P
GB0 = 49                # tiles per PSUM group (49*10*4B = 1960B < 2KB bank)


def build_l0():
    nc = bacc.Bacc(None)
    xT = nc.dram_tensor("xT", [36, NC_PAD], BF16, kind="ExternalInput")
    W1 = nc.dram_tensor("W1", [36, 8], F32, kind="ExternalInput")
    W1TA = nc.dram_tensor("W1TA", [8, 38], F32, kind="ExternalInput")  # [W1T|a1s|a1d]
    HOUT = nc.dram_tensor("HOUT", [P, NT0 * 8], BF16, kind="ExternalOutput")
    SOUT = nc.dram_tensor("SOUT", [P, NT0 * 2], F32, kind="ExternalOutput")

    with tile.TileContext(nc) as tc:
        with (
            tc.tile_pool(name="cst", bufs=1) as cst,
            tc.tile_pool(name="sb", bufs=2) as sb,
            tc.tile_pool(name="ps", bufs=2, space="PSUM") as ps,
            tc.tile_pool(name="big", bufs=2) as big,
        ):
            w1 = cst.tile([36, 8], F32)
            w1ta = cst.tile([8, 38], F32)
            nc.sync.dma_start(out=w1[:], in_=W1[:])
            nc.sync.dma_start(out=w1ta[:], in_=W1TA[:])
            wsd_ps = ps.tile([36, 2], F32, tag="wsd")
            nc.tensor.matmul(wsd_ps[:], lhsT=w1ta[:, 0:36], rhs=w1ta[:, 36:38],
                             start=True, stop=True)
            wcat = cst.tile([36, 10], BF16)
            nc.vector.tensor_copy(out=wcat[:, 0:8], in_=w1[:])
            nc.vector.tensor_copy(out=wcat[:, 8:10], in_=wsd_ps[:])

            for g in range(0, NT0, GB0):
                gn = min(GB0, NT0 - g)
                xt = sb.tile([36, GB0 * P], BF16, tag="xt")
                nc.sync.dma_start(out=xt[:, : gn * P],
                                  in_=xT[:, g * P : (g + gn) * P])
                pst = ps.tile([P, GB0, 10], F32, tag="pst")
                for k in range(gn):
                    nc.tensor.matmul(pst[:, k, :],
                                     lhsT=xt[:, k * P : (k + 1) * P],
                                     rhs=wcat[:], start=True, stop=True)
                hs = big.tile([P, GB0, 8], BF16, tag="hs")
                ss = big.tile([P, GB0, 2], F32, tag="ss")
                nc.vector.tensor_copy(out=hs[:, :gn, :], in_=pst[:, :gn, 0:8])
                nc.vector.tensor_copy(out=ss[:, :gn, :], in_=pst[:, :gn, 8:10])
                nc.sync.dma_start(
                    out=HOUT[:, g * 8 : (g + gn) * 8],
                    in_=hs[:, :gn].rearrange("p a b -> p (a b)"))
                nc.sync.dma_start(
                    out=SOUT[:, g * 2 : (g + gn) * 2],
                    in_=ss[:, :gn].rearrange("p a b -> p (a b)"))
    nc.finalize()
    return nc


# ----------------------------------------------------------------------
# Launch 1 / Launch 2 shared reduction kernel
# ----------------------------------------------------------------------
CHUNK1 = 1792           # slots per partition per chunk (layer 1)
CHUNK2 = 1536           # slots per partition per chunk (layer 2)


def build_reduce_layer(plan, Fh, layer):
    """layer==1: h1 agg + out1 + h2/s2 tail. layer==2: h2 agg + log_softmax.
    One merged stream SH [P, sum (Fh+1)*D*nb]: rows 0:Fh features, row Fh
    s_src. TB [P, n_pb] s_dst. Row Fh is rewritten in place to
    w = exp(leaky_relu(s_src+s_dst, 0.2)) so z folds into the add-tree."""
    buckets = plan["buckets"]
    n_pb = plan["n_pb"]
    S_part = plan["S_part"]
    FT = Fh + 1
    LH = sum(Fh * D * nb for D, nb in buckets)
    chunk = CHUNK1 if layer == 1 else CHUNK2

    nc = bacc.Bacc(None)
    SA = nc.dram_tensor("SA", [P, S_part], BF16, kind="ExternalInput")
    SH = nc.dram_tensor("SH", [P, LH], BF16, kind="ExternalInput")
    TB = nc.dram_tensor("TB", [P, n_pb], BF16, kind="ExternalInput")
    BIAS = nc.dram_tensor("BIAS", [P, Fh], F32, kind="ExternalInput")
    if layer == 1:
        W2R = nc.dram_tensor("W2R", [P, 2 * 8], F32, kind="ExternalInput")
        A2V = nc.dram_tensor("A2V", [P, 2 * 2], F32, kind="ExternalInput")
        TOUTH = nc.dram_tensor("TOUTH", [P, n_pb * 2], BF16, kind="ExternalOutput")
        TOUTS = nc.dram_tensor("TOUTS", [P, n_pb * 2], F32, kind="ExternalOutput")
    else:
        TOUT2 = nc.dram_tensor("TOUT2", [P, n_pb * 2], F32, kind="ExternalOutput")

    with tile.TileContext(nc) as tc:
        with (
            tc.tile_pool(name="cst", bufs=1) as cst,
            tc.tile_pool(name="acc", bufs=1) as accp,
            tc.tile_pool(name="sa", bufs=4 if layer == 1 else 8) as sap,
            tc.tile_pool(name="wp", bufs=6) as wpp,
            tc.tile_pool(name="sh", bufs=4 if layer == 1 else 8) as shp,
        ):
            tb = cst.tile([P, n_pb, 1], BF16)
            nc.sync.dma_start(out=tb[:, :, 0], in_=TB[:])
            bias = cst.tile([P, 1, Fh], F32)
            gacc = accp.tile([P, n_pb, FT], F32)

            # tail tiles + emitter: the first half runs while later buckets
            # are still reducing, hiding most of the serial epilogue
            rz = accp.tile([P, n_pb, 1], F32)
            out = accp.tile([P, n_pb, Fh], F32)
            if layer == 1:
                w2r = cst.tile([P, 2, 8], F32)
                a2v = cst.tile([P, 2, 2], F32)
                h2in = accp.tile([P, n_pb, 8], F32)
                h2 = accp.tile([P, n_pb, 2], F32)
                tmp = accp.tile([P, n_pb, 8], F32)
                tmpg = accp.tile([P, n_pb, 8], F32)
                s2 = accp.tile([P, n_pb, 2], F32)
                tmp2 = accp.tile([P, n_pb, 2], F32)
                outh = accp.tile([P, n_pb, 2], BF16)
            else:
                tm = accp.tile([P, n_pb, 1], F32)
                tt2 = accp.tile([P, n_pb, 2], F32)
                te = accp.tile([P, n_pb, 2], F32)
                tss = accp.tile([P, n_pb, 1], F32)
                tls = accp.tile([P, n_pb, 1], F32)
                res = accp.tile([P, n_pb, 2], F32)
            tail_upto = [0]

            def emit_tail(b0, b1):
                if b0 >= b1:
                    return
                n = b1 - b0
                s = slice(b0, b1)
                if b0 == 0:
                    nc.sync.dma_start(out=bias[:, 0, :], in_=BIAS[:])
                    if layer == 1:
                        nc.sync.dma_start(
                            out=w2r[:],
                            in_=W2R[:].rearrange("p (a b) -> p a b", a=2))
                        nc.sync.dma_start(
                            out=a2v[:],
                            in_=A2V[:].rearrange("p (a b) -> p a b", a=2))
                nc.vector.tensor_scalar_add(out=rz[:, s, 0], in0=gacc[:, s, Fh],
                                            scalar1=1e-16)
                nc.vector.reciprocal(out=rz[:, s, 0], in_=rz[:, s, 0])
                nc.vector.tensor_tensor(
                    out=out[:, s], in0=gacc[:, s, 0:Fh],
                    in1=rz[:, s].to_broadcast([P, n, Fh]), op=ALU.mult)
                nc.vector.tensor_tensor(
                    out=out[:, s], in0=out[:, s],
                    in1=bias[:].to_broadcast([P, n, Fh]), op=ALU.add)
                if layer == 1:
                    nc.vector.tensor_relu(out=h2in[:, s], in_=out[:, s])
                    for c, eng, tt in ((0, nc.vector, tmp), (1, nc.gpsimd, tmpg)):
                        eng.tensor_tensor(
                            out=tt[:, s], in0=h2in[:, s],
                            in1=w2r[:, c : c + 1, :].to_broadcast([P, n, 8]),
                            op=ALU.mult)
                        nc.vector.tensor_reduce(out=h2[:, s, c : c + 1],
                                                in_=tt[:, s], axis=AX.X, op=ALU.add)
                    for c in range(2):
                        nc.vector.tensor_tensor(
                            out=tmp2[:, s], in0=h2[:, s],
                            in1=a2v[:, c : c + 1, :].to_broadcast([P, n, 2]),
                            op=ALU.mult)
                        nc.vector.tensor_reduce(out=s2[:, s, c : c + 1],
                                                in_=tmp2[:, s], axis=AX.X, op=ALU.add)
                    nc.vector.tensor_copy(out=outh[:, s], in_=h2[:, s])
                    nc.sync.dma_start(
                        out=TOUTH[:, b0 * 2 : b1 * 2],
                        in_=outh[:, s].rearrange("p a b -> p (a b)"))
                    nc.sync.dma_start(
                        out=TOUTS[:, b0 * 2 : b1 * 2],
                        in_=s2[:, s].rearrange("p a b -> p (a b)"))
                else:
                    nc.vector.tensor_tensor(out=tm[:, s], in0=out[:, s, 0:1],
                                            in1=out[:, s, 1:2], op=ALU.max)
                    nc.vector.tensor_tensor(
                        out=tt2[:, s], in0=out[:, s],
                        in1=tm[:, s].to_broadcast([P, n, 2]), op=ALU.subtract)
                    nc.scalar.activation(te[:, s], tt2[:, s], AF.Exp)
                    nc.vector.tensor_tensor(out=tss[:, s], in0=te[:, s, 0:1],
                                            in1=te[:, s, 1:2], op=ALU.add)
                    nc.scalar.activation(tls[:, s], tss[:, s], AF.Ln)
                    nc.vector.tensor_tensor(
                        out=res[:, s], in0=tt2[:, s],
                        in1=tls[:, s].to_broadcast([P, n, 2]), op=ALU.subtract)
                    nc.sync.dma_start(
                        out=TOUT2[:, b0 * 2 : b1 * 2],
                        in_=res[:, s].rearrange("p a b -> p (a b)"))

            def tree4(eng, t4, f0, f1, D, stop=8):
                d = D
                while d > stop:
                    h = d // 2
                    eng.tensor_tensor(
                        out=t4[:, :, f0:f1, 0:h], in0=t4[:, :, f0:f1, 0:h],
                        in1=t4[:, :, f0:f1, h : 2 * h], op=ALU.add)
                    if d % 2:
                        eng.tensor_tensor(
                            out=t4[:, :, f0:f1, 0:1], in0=t4[:, :, f0:f1, 0:1],
                            in1=t4[:, :, f0:f1, d - 1 : d], op=ALU.add)
                    d = h
                return d

            h_off = 0
            a_off = 0
            nb_off = 0
            for D, nb in buckets:
                n_ch = max(1, -(-(nb * D) // chunk))
                cb = -(-nb // n_ch)
                for j0 in range(0, nb, cb):
                    cbn = min(cb, nb - j0)
                    blks = slice(nb_off + j0, nb_off + j0 + cbn)
                    sa = sap.tile([P, cb, D], BF16, tag="sa")
                    nc.sync.dma_start(
                        out=sa[:, :cbn],
                        in_=SA[:, a_off + j0 * D : a_off + (j0 + cbn) * D]
                        .rearrange("p (c d) -> p c d", d=D))
                    if layer == 1:
                        # w lives as feature row Fh of q: z rides the q tree
                        q = shp.tile([P, cb, FT, D], BF16, tag="q")
                        nc.sync.dma_start(
                            out=q[:, :cbn, 0:Fh, :],
                            in_=SH[:, h_off + j0 * Fh * D : h_off + (j0 + cbn) * Fh * D]
                            .rearrange("p (c k d) -> p c k d", k=Fh, d=D))
                        wv = q[:, :cbn, Fh:FT, :]
                    else:
                        # separate w tile keeps the SH DMA fully contiguous
                        q = shp.tile([P, cb, Fh, D], BF16, tag="q")
                        nc.sync.dma_start(
                            out=q[:, :cbn],
                            in_=SH[:, h_off + j0 * Fh * D : h_off + (j0 + cbn) * Fh * D]
                            .rearrange("p (c k d) -> p c k d", k=Fh, d=D))
                        w = wpp.tile([P, cb, 1, D], BF16, tag="w")
                        wv = w[:, :cbn]
                    # e = s_src + s_dst (Pool); w = exp(leaky_relu(e, .2)) (ACT)
                    nc.gpsimd.tensor_tensor(
                        out=sa[:, :cbn, :], in0=sa[:, :cbn, :],
                        in1=tb[:, blks, :].to_broadcast([P, cbn, D]),
                        op=ALU.add)
                    nc.scalar.activation(sa[:, :cbn, :], sa[:, :cbn, :],
                                         AF.Prelu, alpha=0.2)
                    nc.scalar.activation(wv[:, :, 0, :], sa[:, :cbn, :],
                                         AF.Exp)

                    if layer == 1:
                        # mults: DVE f0:7, Pool f7; trees: DVE f0:7, Pool f7:9
                        nc.vector.tensor_tensor(
                            out=q[:, :cbn, 0:7, :], in0=q[:, :cbn, 0:7, :],
                            in1=wv.to_broadcast([P, cbn, 7, D]), op=ALU.mult)
                        nc.gpsimd.tensor_tensor(
                            out=q[:, :cbn, 7:8, :], in0=q[:, :cbn, 7:8, :],
                            in1=wv.to_broadcast([P, cbn, 1, D]), op=ALU.mult)
                        d = tree4(nc.vector, q[:, :cbn], 0, 7, D)
                        tree4(nc.gpsimd, q[:, :cbn], 7, FT, D)
                        nc.vector.tensor_reduce(
                            out=gacc[:, blks, :],
                            in_=q[:, :cbn, :, 0:d], axis=AX.X, op=ALU.add)
                    else:
                        # mults/feature-trees on DVE; w(z)-tree on Pool
                        nc.vector.tensor_tensor(
                            out=q[:, :cbn, :, :], in0=q[:, :cbn, :, :],
                            in1=wv.to_broadcast([P, cbn, Fh, D]), op=ALU.mult)
                        d = tree4(nc.vector, q[:, :cbn], 0, Fh, D)
                        dw = tree4(nc.vector, w[:, :cbn], 0, 1, D)
                        nc.vector.tensor_reduce(
                            out=gacc[:, blks, 0:Fh],
                            in_=q[:, :cbn, :, 0:d], axis=AX.X, op=ALU.add)
                        nc.vector.tensor_reduce(
                            out=gacc[:, blks, Fh:FT],
                            in_=w[:, :cbn, :, 0:dw], axis=AX.X, op=ALU.add)
                h_off += nb * Fh * D
                a_off += nb * D
                nb_off += nb

            emit_tail(tail_upto[0], n_pb)
    nc.finalize()
    return nc


# ----------------------------------------------------------------------
# Orchestration
# ----------------------------------------------------------------------

def run_gat(x, W1, a1_src, a1_dst, b1, W2, a2_src, a2_dst, b2, edge_index):
    N = x.shape[0]
    src = np.asarray(edge_index[0], np.int64)
    dst = np.asarray(edge_index[1], np.int64)
    plan = build_plan(dst, N)
    n_pb = plan["n_pb"]
    cores = list(range(N_CORES))

    # ---------------- L0 ----------------
    nc0 = build_l0()
    xpad = np.zeros((N_CORES, NC_PAD, 36), bf16)
    xpad[:, :NC_NODES] = np.asarray(x).reshape(N_CORES, NC_NODES, 36).astype(bf16)
    W1f = np.asarray(W1, np.float32)
    AVf = np.stack([np.asarray(a1_src, np.float32), np.asarray(a1_dst, np.float32)], 1)
    in_maps0 = []
    W1TA = np.ascontiguousarray(np.concatenate([W1f.T, AVf], axis=1))
    for c in cores:
        in_maps0.append({
            "xT": np.ascontiguousarray(xpad[c].T),
            "W1": W1f, "W1TA": W1TA,
        })
    r0 = run_bass_kernel_spmd(nc0, in_maps0, cores).results
    h1bf = np.zeros((N, 8), bf16)
    s1s = np.zeros((N,), np.float32)
    s1d = np.zeros((N,), np.float32)
    for c in cores:
        # node t*P+p lives at [p, t, :]
        hcore = np.asarray(r0[c]["HOUT"]).reshape(P, NT0, 8).transpose(1, 0, 2)
        score = np.asarray(r0[c]["SOUT"]).reshape(P, NT0, 2).transpose(1, 0, 2)
        sl = slice(c * NC_NODES, (c + 1) * NC_NODES)
        h1bf[sl] = hcore.reshape(NC_PAD, 8)[:NC_NODES]
        s = score.reshape(NC_PAD, 2)[:NC_NODES]
        s1s[sl] = s[:, 0]
        s1d[sl] = s[:, 1]

    # ---------------- L1 ----------------
    nc1 = build_reduce_layer(plan, 8, 1)
    b1rep = np.tile(np.asarray(b1, np.float32)[None, :], (P, 1))
    W2rep = np.tile(np.asarray(W2, np.float32).T.reshape(1, 2, 8), (P, 1, 1)).reshape(P, 16)
    A2Vrep = np.tile(np.stack([np.asarray(a2_src, np.float32), np.asarray(a2_dst, np.float32)], 0).reshape(1, 2, 2), (P, 1, 1)).reshape(P, 4)
    in_maps1 = []
    for c in cores:
        SAc, SHc = build_streams(plan, c, src, s1s, h1bf)
        TB = build_node_tab(plan, c, s1d[:, None], 1).reshape(P, -1)
        in_maps1.append({
            "SA": SAc, "SH": SHc, "TB": TB, "BIAS": b1rep,
            "W2R": W2rep, "A2V": A2Vrep,
        })
    r1 = run_bass_kernel_spmd(nc1, in_maps1, cores).results

    # host: rebuild global tables for L2
    h2bf = np.zeros((N, 2), bf16)
    s2s = np.zeros((N,), np.float32)
    s2d = np.zeros((N,), np.float32)
    for c in cores:
        no = plan["cores"][c]["node_order"].reshape(n_pb, P)
        valid = no >= 0
        outh = np.asarray(r1[c]["TOUTH"]).reshape(P, n_pb, 2).transpose(1, 0, 2)
        outs = np.asarray(r1[c]["TOUTS"]).reshape(P, n_pb, 2).transpose(1, 0, 2)
        h2bf[no[valid]] = outh[valid]
        s2s[no[valid]] = outs[valid][:, 0]
        s2d[no[valid]] = outs[valid][:, 1]

    # ---------------- L2 ----------------
    nc2 = build_reduce_layer(plan, 2, 2)
    b2rep = np.tile(np.asarray(b2, np.float32)[None, :], (P, 1))
    in_maps2 = []
    for c in cores:
        SAc, SHc = build_streams(plan, c, src, s2s, h2bf)
        TB = build_node_tab(plan, c, s2d[:, None], 1).reshape(P, -1)
        in_maps2.append({"SA": SAc, "SH": SHc, "TB": TB, "BIAS": b2rep})
    r2 = run_bass_kernel_spmd(nc2, in_maps2, cores).results

    out = np.zeros((N, 2), np.float32)
    for c in cores:
        no = plan["cores"][c]["node_order"].reshape(n_pb, P)
        valid = no >= 0
        o = np.asarray(r2[c]["TOUT2"]).reshape(P, n_pb, 2).transpose(1, 0, 2)
        out[no[valid]] = o[valid]
    return out


def kernel(**inputs):
    out = run_gat(
        x=np.asarray(inputs["x"], np.float32),
        W1=np.asarray(inputs["W1"], np.float32),
        a1_src=np.asarray(inputs["a1_src"], np.float32),
        a1_dst=np.asarray(inputs["a1_dst"], np.float32),
        b1=np.asarray(inputs["b1"], np.float32),
        W2=np.asarray(inputs["W2"], np.float32),
        a2_src=np.asarray(inputs["a2_src"], np.float32),
        a2_dst=np.asarray(inputs["a2_dst"], np.float32),
        b2=np.asarray(inputs["b2"], np.float32),
        edge_index=np.asarray(inputs["edge_index"]),
    )
    return out.astype(np.float32)


# revision 65
# speedup vs baseline: 1.0204x; 1.0204x over previous
"""Two-layer GAT (GATConv x2 + log_softmax) on 8 Trainium2 NeuronCores.

Strategy (edge-parallel by dst node, zero on-device gathers):
  Host does pure index relabeling (argsort / np.take / dtype casts); all
  FLOPs run on-device across 3 SPMD launches.

  L0: nodes-on-partitions. One bf16 matmul per 128-node tile against
      Wcat = [W1 | W1@a_src | W1@a_dst] (built on device) yields h1 and
      both attention scores s_src/s_dst in a single pass.
  host: edges sorted by dst; per-node degree-bucketed padded slot layout
      (granularity 8), buckets dealt round-robin across the 8 cores so
      every core gets an identical block layout; per-slot streams:
      s_src (1 bf16) + h1 rows (8 bf16), dummy slots get s_src = -100
      so their softmax weight underflows to ~0.
  L1: e = s_src + s_dst (DVE/Pool), w = Exp(Prelu(e, 0.2)) on the ACT
      engine written straight into a 9th feature row of the q tile, so
      the z = sum(w) rides the same halving add-tree as the weighted
      feature sums. mults/trees split across DVE+GPSIMD by a per-chunk
      schedule; out1 = agg/(z+1e-16)+b1, relu, h2 = @W2, s2 scores.
  host: same relabeling for layer-2 streams (h2 + s2 scores).
  L2: same reduction with 2 features + w row, then log_softmax (f32).
"""

import numpy as np
import ml_dtypes

import concourse.bass as bass
import concourse.bacc as bacc
import concourse.mybir as mybir
import concourse.tile as tile
from concourse.bass_utils import run_bass_kernel_spmd

bf16 = ml_dtypes.bfloat16
P = 128
N_CORES = 8
GRAN = 8
SENTINEL = -100.0
F32 = mybir.dt.float32
BF16 = mybir.dt.bfloat16
AF = mybir.ActivationFunctionType
ALU = mybir.AluOpType
AX = mybir.AxisListType


# ----------------------------------------------------------------------
# Host-side plan (index arithmetic only)
# ----------------------------------------------------------------------

def build_plan(dst, n_nodes):
    E = dst.shape[0]
    order = np.argsort(dst, kind="stable").astype(np.int64)
    deg = np.bincount(dst, minlength=n_nodes).astype(np.int64)
    starts = np.zeros(n_nodes + 1, np.int64)
    np.cumsum(deg, out=starts[1:])

    Dpad = np.maximum(((deg + GRAN - 1) // GRAN) * GRAN, GRAN)
    Ds = np.unique(Dpad)

    # deal each bucket's nodes round-robin across cores: per-core counts are
    # equal (+-1) so the unified block counts carry no cross-core padding
    nblk = {}
    percore_nodes = [dict() for _ in range(N_CORES)]
    for D in Ds:
        sel = np.nonzero(Dpad == D)[0]
        for c in range(N_CORES):
            mine = sel[c::N_CORES]
            percore_nodes[c][int(D)] = mine
            nb = (len(mine) + P - 1) // P
            nblk[int(D)] = max(nblk.get(int(D), 0), nb)
    buckets = [(int(D), nblk[int(D)]) for D in Ds if nblk[int(D)] > 0][::-1]
    bounds = None

    n_pb = sum(nb for _, nb in buckets)             # node-blocks per partition
    S_part = sum(D * nb for D, nb in buckets)       # slots per partition

    cores = []
    for c in range(N_CORES):
        node_order = []
        for D, nb in buckets:
            sel = percore_nodes[c][D]
            padded = np.full(nb * P, -1, np.int64)
            padded[: len(sel)] = sel
            node_order.append(padded)
        node_order = np.concatenate(node_order)      # [(bucket,block,partition)]
        # perm: [P, S_part] edge index or -1, laid out per partition as
        # concat over buckets of [nblk, D]
        perm = np.full((P, S_part), -1, np.int64)
        no = node_order.reshape(n_pb, P)
        off = 0
        bi = 0
        for D, nb in buckets:
            for j in range(nb):
                nid = no[bi + j]                      # [P]
                for p in range(P):
                    n = nid[p]
                    if n < 0:
                        continue
                    s0 = starts[n]
                    cdeg = deg[n]
                    perm[p, off + j * D : off + j * D + cdeg] = order[s0 : s0 + cdeg]
            off += nb * D
            bi += nb
        cores.append({"node_order": node_order, "perm": perm})

    return {
        "buckets": buckets,
        "n_pb": n_pb,
        "S_part": S_part,
        "bounds": bounds,
        "cores": cores,
    }


def build_streams(plan, core_idx, src, s_tab, H_tab):
    """s_tab [N] f32 (src attention score), H_tab [N,F] bf16. Returns
    SA bf16 [P, S_part] flat s_src per slot (dummy slots = SENTINEL so
    their softmax weight underflows to ~0) and SH bf16
    [P, sum F*D*nblk] with per-bucket layout [nblk, F, D] per part."""
    core = plan["cores"][core_idx]
    perm = core["perm"]
    valid = perm >= 0
    Fh = H_tab.shape[1]
    src_slot = np.where(valid, src[np.clip(perm, 0, None)], 0)
    Aslot = s_tab[src_slot].astype(np.float32)   # [P, S]
    Aslot[~valid] = SENTINEL
    Hslot = H_tab[src_slot]                      # [P, S, F]
    Hslot[~valid] = 0
    parts = []
    off = 0
    for D, nb in plan["buckets"]:
        n = nb * D
        h = Hslot[:, off : off + n].reshape(P, nb, D, Fh).transpose(0, 1, 3, 2)
        parts.append(np.ascontiguousarray(h).reshape(P, -1).astype(bf16))
        off += n
    return Aslot.astype(bf16), np.concatenate(parts, axis=1)


def build_tbe(plan, core_idx, tab):
    """tab [N] f32 -> [P, S_part] bf16: per-slot s_dst (node value repeated
    across its D slots; zeros for dummy nodes)."""
    core = plan["cores"][core_idx]
    no = core["node_order"].reshape(plan["n_pb"], P)
    valid = no >= 0
    t = tab[np.clip(no, 0, None)].copy()         # [n_pb, P]
    t[~valid] = 0.0
    parts = []
    nbo = 0
    for D, nb in plan["buckets"]:
        blk = t[nbo : nbo + nb].T                # [P, nb]
        parts.append(np.repeat(blk[:, :, None], D, axis=2).reshape(P, nb * D))
        nbo += nb
    return np.concatenate(parts, axis=1).astype(bf16)


def build_node_tab(plan, core_idx, tab, k):
    """tab [N,k] f32 -> [P, n_pb, k] per layout order (zeros for dummies)."""
    core = plan["cores"][core_idx]
    no = core["node_order"].reshape(plan["n_pb"], P)
    valid = no >= 0
    t = tab[np.clip(no, 0, None)].copy()         # [n_pb, P, k]
    t[~valid] = 0.0
    return np.ascontiguousarray(t.transpose(1, 0, 2)).astype(bf16)


# ----------------------------------------------------------------------
# Launch 0: h1 = x@W1, s_src/s_dst scores - nodes on partitions
# ----------------------------------------------------------------------
NC_NODES = 12500
NC_PAD = 12544          # 98 tiles of 128
NT0 = NC_PAD // P
GB0 = 49                # tiles per PSUM group (49*10*4B = 1960B < 2KB bank)


def build_l0():
    nc = bacc.Bacc(None)
    xT = nc.dram_tensor("xT", [36, NC_PAD], BF16, kind="ExternalInput")
    W1 = nc.dram_tensor("W1", [36, 8], F32, kind="ExternalInput")
    W1TA = nc.dram_tensor("W1TA", [8, 38], F32, kind="ExternalInput")  # [W1T|a1s|a1d]
    HOUT = nc.dram_tensor("HOUT", [P, NT0 * 8], BF16, kind="ExternalOutput")
    SOUT = nc.dram_tensor("SOUT", [P, NT0 * 2], F32, kind="ExternalOutput")

    with tile.TileContext(nc) as tc:
        with (
            tc.tile_pool(name="cst", bufs=1) as cst,
            tc.tile_pool(name="sb", bufs=2) as sb,
            tc.tile_pool(name="ps", bufs=2, space="PSUM") as ps,
            tc.tile_pool(name="big", bufs=2) as big,
        ):
            w1 = cst.tile([36, 8], F32)
            w1ta = cst.tile([8, 38], F32)
            nc.sync.dma_start(out=w1[:], in_=W1[:])
            nc.sync.dma_start(out=w1ta[:], in_=W1TA[:])
            wsd_ps = ps.tile([36, 2], F32, tag="wsd")
            nc.tensor.matmul(wsd_ps[:], lhsT=w1ta[:, 0:36], rhs=w1ta[:, 36:38],
                             start=True, stop=True)
            wcat = cst.tile([36, 10], BF16)
            nc.vector.tensor_copy(out=wcat[:, 0:8], in_=w1[:])
            nc.vector.tensor_copy(out=wcat[:, 8:10], in_=wsd_ps[:])

            for g in range(0, NT0, GB0):
                gn = min(GB0, NT0 - g)
                xt = sb.tile([36, GB0 * P], BF16, tag="xt")
                nc.sync.dma_start(out=xt[:, : gn * P],
                                  in_=xT[:, g * P : (g + gn) * P])
                pst = ps.tile([P, GB0, 10], F32, tag="pst")
                for k in range(gn):
                    nc.tensor.matmul(pst[:, k, :],
                                     lhsT=xt[:, k * P : (k + 1) * P],
                                     rhs=wcat[:], start=True, stop=True)
                hs = big.tile([P, GB0, 8], BF16, tag="hs")
                ss = big.tile([P, GB0, 2], F32, tag="ss")
                nc.vector.tensor_copy(out=hs[:, :gn, :], in_=pst[:, :gn, 0:8])
                nc.vector.tensor_copy(out=ss[:, :gn, :], in_=pst[:, :gn, 8:10])
                nc.sync.dma_start(
                    out=HOUT[:, g * 8 : (g + gn) * 8],
                    in_=hs[:, :gn].rearrange("p a b -> p (a b)"))
                nc.sync.dma_start(
                    out=SOUT[:, g * 2 : (g + gn) * 2],
                    in_=ss[:, :gn].rearrange("p a b -> p (a b)"))
    nc.finalize()
    return nc


# ----------------------------------------------------------------------
# Launch 1 / Launch 2 shared reduction kernel
# ----------------------------------------------------------------------
CHUNK1 = 1792           # slots per partition per chunk (layer 1)
CHUNK2 = 1536           # slots per partition per chunk (layer 2)


def build_reduce_layer(plan, Fh, layer):
    """layer==1: h1 agg + out1 + h2/s2 tail. layer==2: h2 agg + log_softmax.
    One merged stream SH [P, sum (Fh+1)*D*nb]: rows 0:Fh features, row Fh
    s_src. TB [P, n_pb] s_dst. Row Fh is rewritten in place to
    w = exp(leaky_relu(s_src+s_dst, 0.2)) so z folds into the add-tree."""
    buckets = plan["buckets"]
    n_pb = plan["n_pb"]
    S_part = plan["S_part"]
    FT = Fh + 1
    LH = sum(Fh * D * nb for D, nb in buckets)
    chunk = CHUNK1 if layer == 1 else CHUNK2

    nc = bacc.Bacc(None)
    SA = nc.dram_tensor("SA", [P, S_part], BF16, kind="ExternalInput")
    SH = nc.dram_tensor("SH", [P, LH], BF16, kind="ExternalInput")
    TB = nc.dram_tensor("TB", [P, n_pb], BF16, kind="ExternalInput")
    BIAS = nc.dram_tensor("BIAS", [P, Fh], F32, kind="ExternalInput")
    if layer == 1:
        W2R = nc.dram_tensor("W2R", [P, 2 * 8], F32, kind="ExternalInput")
        A2V = nc.dram_tensor("A2V", [P, 2 * 2], F32, kind="ExternalInput")
        TOUTH = nc.dram_tensor("TOUTH", [P, n_pb * 2], BF16, kind="ExternalOutput")
        TOUTS = nc.dram_tensor("TOUTS", [P, n_pb * 2], F32, kind="ExternalOutput")
    else:
        TOUT2 = nc.dram_tensor("TOUT2", [P, n_pb * 2], F32, kind="ExternalOutput")

    with tile.TileContext(nc) as tc:
        with (
            tc.tile_pool(name="cst", bufs=1) as cst,
            tc.tile_pool(name="acc", bufs=1) as accp,
            tc.tile_pool(name="sa", bufs=4 if layer == 1 else 8) as sap,
            tc.tile_pool(name="wp", bufs=6) as wpp,
            tc.tile_pool(name="sh", bufs=4 if layer == 1 else 8) as shp,
        ):
            tb = cst.tile([P, n_pb, 1], BF16)
            nc.sync.dma_start(out=tb[:, :, 0], in_=TB[:])
            bias = cst.tile([P, 1, Fh], F32)
            gacc = accp.tile([P, n_pb, FT], F32)

            # tail tiles + emitter: the first half runs while later buckets
            # are still reducing, hiding most of the serial epilogue
            rz = accp.tile([P, n_pb, 1], F32)
            out = accp.tile([P, n_pb, Fh], F32)
            if layer == 1:
                w2r = cst.tile([P, 2, 8], F32)
                a2v = cst.tile([P, 2, 2], F32)
                h2in = accp.tile([P, n_pb, 8], F32)
                h2 = accp.tile([P, n_pb, 2], F32)
                tmp = accp.tile([P, n_pb, 8], F32)
                tmpg = accp.tile([P, n_pb, 8], F32)
                s2 = accp.tile([P, n_pb, 2], F32)
                tmp2 = accp.tile([P, n_pb, 2], F32)
                outh = accp.tile([P, n_pb, 2], BF16)
            else:
                tm = accp.tile([P, n_pb, 1], F32)
                tt2 = accp.tile([P, n_pb, 2], F32)
                te = accp.tile([P, n_pb, 2], F32)
                tss = accp.tile([P, n_pb, 1], F32)
                tls = accp.tile([P, n_pb, 1], F32)
                res = accp.tile([P, n_pb, 2], F32)
            tail_upto = [0]

            def emit_tail(b0, b1):
                if b0 >= b1:
                    return
                n = b1 - b0
                s = slice(b0, b1)
                if b0 == 0:
                    nc.sync.dma_start(out=bias[:, 0, :], in_=BIAS[:])
                    if layer == 1:
                        nc.sync.dma_start(
                            out=w2r[:],
                            in_=W2R[:].rearrange("p (a b) -> p a b", a=2))
                        nc.sync.dma_start(
                            out=a2v[:],
                            in_=A2V[:].rearrange("p (a b) -> p a b", a=2))
                nc.vector.tensor_scalar_add(out=rz[:, s, 0], in0=gacc[:, s, Fh],
                                            scalar1=1e-16)
                nc.vector.reciprocal(out=rz[:, s, 0], in_=rz[:, s, 0])
                nc.vector.tensor_tensor(
                    out=out[:, s], in0=gacc[:, s, 0:Fh],
                    in1=rz[:, s].to_broadcast([P, n, Fh]), op=ALU.mult)
                nc.vector.tensor_tensor(
                    out=out[:, s], in0=out[:, s],
                    in1=bias[:].to_broadcast([P, n, Fh]), op=ALU.add)
                if layer == 1:
                    nc.vector.tensor_relu(out=h2in[:, s], in_=out[:, s])
                    for c, eng, tt in ((0, nc.vector, tmp), (1, nc.gpsimd, tmpg)):
                        eng.tensor_tensor(
                            out=tt[:, s], in0=h2in[:, s],
                            in1=w2r[:, c : c + 1, :].to_broadcast([P, n, 8]),
                            op=ALU.mult)
                        nc.vector.tensor_reduce(out=h2[:, s, c : c + 1],
                                                in_=tt[:, s], axis=AX.X, op=ALU.add)
                    for c in range(2):
                        nc.vector.tensor_tensor(
                            out=tmp2[:, s], in0=h2[:, s],
                            in1=a2v[:, c : c + 1, :].to_broadcast([P, n, 2]),
                            op=ALU.mult)
                        nc.vector.tensor_reduce(out=s2[:, s, c : c + 1],
                                                in_=tmp2[:, s], axis=AX.X, op=ALU.add)
                    nc.vector.tensor_copy(out=outh[:, s], in_=h2[:, s])
                    nc.sync.dma_start(
                        out=TOUTH[:, b0 * 2 : b1 * 2],
                        in_=outh[:, s].rearrange("p a b -> p (a b)"))
                    nc.sync.dma_start(
                        out=TOUTS[:, b0 * 2 : b1 * 2],
                        in_=s2[:, s].rearrange("p a b -> p (a b)"))
                else:
                    nc.vector.tensor_tensor(out=tm[:, s], in0=out[:, s, 0:1],
                                            in1=out[:, s, 1:2], op=ALU.max)
                    nc.vector.tensor_tensor(
                        out=tt2[:, s], in0=out[:, s],
                        in1=tm[:, s].to_broadcast([P, n, 2]), op=ALU.subtract)
                    nc.scalar.activation(te[:, s], tt2[:, s], AF.Exp)
                    nc.vector.tensor_tensor(out=tss[:, s], in0=te[:, s, 0:1],
                                            in1=te[:, s, 1:2], op=ALU.add)
                    nc.scalar.activation(tls[:, s], tss[:, s], AF.Ln)
                    nc.vector.tensor_tensor(
                        out=res[:, s], in0=tt2[:, s],
                        in1=tls[:, s].to_broadcast([P, n, 2]), op=ALU.subtract)
                    nc.sync.dma_start(
                        out=TOUT2[:, b0 * 2 : b1 * 2],
                        in_=res[:, s].rearrange("p a b -> p (a b)"))

            def tree4(eng, t4, f0, f1, D, stop=8):
                d = D
                while d > stop:
                    h = d // 2
                    eng.tensor_tensor(
                        out=t4[:, :, f0:f1, 0:h], in0=t4[:, :, f0:f1, 0:h],
                        in1=t4[:, :, f0:f1, h : 2 * h], op=ALU.add)
                    if d % 2:
                        eng.tensor_tensor(
                            out=t4[:, :, f0:f1, 0:1], in0=t4[:, :, f0:f1, 0:1],
                            in1=t4[:, :, f0:f1, d - 1 : d], op=ALU.add)
                    d = h
                return d

            h_off = 0
            a_off = 0
            nb_off = 0
            for D, nb in buckets:
                n_ch = max(1, -(-(nb * D) // chunk))
                cb = -(-nb // n_ch)
                for j0 in range(0, nb, cb):
                    cbn = min(cb, nb - j0)
                    blks = slice(nb_off + j0, nb_off + j0 + cbn)
                    sa = sap.tile([P, cb, D], BF16, tag="sa")
                    nc.sync.dma_start(
                        out=sa[:, :cbn],
                        in_=SA[:, a_off + j0 * D : a_off + (j0 + cbn) * D]
                        .rearrange("p (c d) -> p c d", d=D))
                    if layer == 1:
                        # w lives as feature row Fh of q: z rides the q tree
                        q = shp.tile([P, cb, FT, D], BF16, tag="q")
                        nc.sync.dma_start(
                            out=q[:, :cbn, 0:Fh, :],
                            in_=SH[:, h_off + j0 * Fh * D : h_off + (j0 + cbn) * Fh * D]
                            .rearrange("p (c k d) -> p c k d", k=Fh, d=D))
                        wv = q[:, :cbn, Fh:FT, :]
                    else:
                        # separate w tile keeps the SH DMA fully contiguous
                        q = shp.tile([P, cb, Fh, D], BF16, tag="q")
                        nc.sync.dma_start(
                            out=q[:, :cbn],
                            in_=SH[:, h_off + j0 * Fh * D : h_off + (j0 + cbn) * Fh * D]
                            .rearrange("p (c k d) -> p c k d", k=Fh, d=D))
                        w = wpp.tile([P, cb, 1, D], BF16, tag="w")
                        wv = w[:, :cbn]
                    # e = s_src + s_dst (Pool); w = exp(leaky_relu(e, .2)) (ACT)
                    nc.gpsimd.tensor_tensor(
                        out=sa[:, :cbn, :], in0=sa[:, :cbn, :],
                        in1=tb[:, blks, :].to_broadcast([P, cbn, D]),
                        op=ALU.add)
                    nc.scalar.activation(sa[:, :cbn, :], sa[:, :cbn, :],
                                         AF.Prelu, alpha=0.2)
                    nc.scalar.activation(wv[:, :, 0, :], sa[:, :cbn, :],
                                         AF.Exp)

                    if layer == 1:
                        # mults: DVE f0:7, Pool f7; trees: DVE f0:7, Pool f7:9
                        nc.vector.tensor_tensor(
                            out=q[:, :cbn, 0:7, :], in0=q[:, :cbn, 0:7, :],
                            in1=wv.to_broadcast([P, cbn, 7, D]), op=ALU.mult)
                        nc.gpsimd.tensor_tensor(
                            out=q[:, :cbn, 7:8, :], in0=q[:, :cbn, 7:8, :],
                            in1=wv.to_broadcast([P, cbn, 1, D]), op=ALU.mult)
                        d = tree4(nc.vector, q[:, :cbn], 0, 7, D)
                        tree4(nc.gpsimd, q[:, :cbn], 7, FT, D)
                        nc.vector.tensor_reduce(
                            out=gacc[:, blks, :],
                            in_=q[:, :cbn, :, 0:d], axis=AX.X, op=ALU.add)
                    else:
                        # mults/feature-trees on DVE; w(z)-tree on Pool
                        nc.vector.tensor_tensor(
                            out=q[:, :cbn, :, :], in0=q[:, :cbn, :, :],
                            in1=wv.to_broadcast([P, cbn, Fh, D]), op=ALU.mult)
                        d = tree4(nc.vector, q[:, :cbn], 0, Fh, D)
                        dw = tree4(nc.vector, w[:, :cbn], 0, 1, D)
                        nc.vector.tensor_reduce(
                            out=gacc[:, blks, 0:Fh],
                            in_=q[:, :cbn, :, 0:d], axis=AX.X, op=ALU.add)
                        nc.vector.tensor_reduce(
                            out=gacc[:, blks, Fh:FT],
                            in_=w[:, :cbn, :, 0:dw], axis=AX.X, op=ALU.add)
                h_off += nb * Fh * D
                a_off += nb * D
                nb_off += nb

            emit_tail(tail_upto[0], n_pb)
    nc.finalize()
    return nc


# ----------------------------------------------------------------------
# Orchestration
# ----------------------------------------------------------------------

def run_gat(x, W1, a1_src, a1_dst, b1, W2, a2_src, a2_dst, b2, edge_index):
    N = x.shape[0]
    src = np.asarray(edge_index[0], np.int64)
    dst = np.asarray(edge_index[1], np.int64)
    plan = build_plan(dst, N)
    n_pb = plan["n_pb"]
    cores = list(range(N_CORES))

    # ---------------- L0 ----------------
    nc0 = build_l0()
    xpad = np.zeros((N_CORES, NC_PAD, 36), bf16)
    xpad[:, :NC_NODES] = np.asarray(x).reshape(N_CORES, NC_NODES, 36).astype(bf16)
    W1f = np.asarray(W1, np.float32)
    AVf = np.stack([np.asarray(a1_src, np.float32), np.asarray(a1_dst, np.float32)], 1)
    in_maps0 = []
    W1TA = np.ascontiguousarray(np.concatenate([W1f.T, AVf], axis=1))
    for c in cores:
        in_maps0.append({
            "xT": np.ascontiguousarray(xpad[c].T),
            "W1": W1f, "W1TA": W1TA,
        })
    r0 = run_bass_kernel_spmd(nc0, in_maps0, cores).results
    h1bf = np.zeros((N, 8), bf16)
    s1s = np.zeros((N,), np.float32)
    s1d = np.zeros((N,), np.float32)
    for c in cores:
        # node t*P+p lives at [p, t, :]
        hcore = np.asarray(r0[c]["HOUT"]).reshape(P, NT0, 8).transpose(1, 0, 2)
        score = np.asarray(r0[c]["SOUT"]).reshape(P, NT0, 2).transpose(1, 0, 2)
        sl = slice(c * NC_NODES, (c + 1) * NC_NODES)
        h1bf[sl] = hcore.reshape(NC_PAD, 8)[:NC_NODES]
        s = score.reshape(NC_PAD, 2)[:NC_NODES]
        s1s[sl] = s[:, 0]
        s1d[sl] = s[:, 1]

    # ---------------- L1 ----------------
    nc1 = build_reduce_layer(plan, 8, 1)
    b1rep = np.tile(np.asarray(b1, np.float32)[None, :], (P, 1))
    W2rep = np.tile(np.asarray(W2, np.float32).T.reshape(1, 2, 8), (P, 1, 1)).reshape(P, 16)
    A2Vrep = np.tile(np.stack([np.asarray(a2_src, np.float32), np.asarray(a2_dst, np.float32)], 0).reshape(1, 2, 2), (P, 1, 1)).reshape(P, 4)
    in_maps1 = []
    for c in cores:
        SAc, SHc = build_streams(plan, c, src, s1s, h1bf)
        TB = build_node_tab(plan, c, s1d[:, None], 1).reshape(P, -1)
        in_maps1.append({
            "SA": SAc, "SH": SHc, "TB": TB, "BIAS": b1rep,
            "W2R": W2rep, "A2V": A2Vrep,
        })
    r1 = run_bass_kernel_spmd(nc1, in_maps1, cores).results

    # host: rebuild global tables for L2
    h2bf = np.zeros((N, 2), bf16)
    s2s = np.zeros((N,), np.float32)
    s2d = np.zeros((N,), np.float32)
    for c in cores:
        no = plan["cores"][c]["node_order"].reshape(n_pb, P)
        valid = no >= 0
        outh = np.asarray(r1[c]["TOUTH"]).reshape(P, n_pb, 2).transpose(1, 0, 2)
        outs = np.asarray(r1[c]["TOUTS"]).reshape(P, n_pb, 2).transpose(1, 0, 2)
        h2bf[no[valid]] = outh[valid]
        s2s[no[valid]] = outs[valid][:, 0]
        s2d[no[valid]] = outs[valid][:, 1]

    # ---------------- L2 ----------------
    nc2 = build_reduce_layer(plan, 2, 2)
    b2rep = np.tile(np.asarray(b2, np.float32)[None, :], (P, 1))
    in_maps2 = []
    for c in cores:
        SAc, SHc = build_streams(plan, c, src, s2s, h2bf)
        TB = build_node_tab(plan, c, s2d[:, None], 1).reshape(P, -1)
        in_maps2.append({"SA": SAc, "SH": SHc, "TB": TB, "BIAS": b2rep})
    r2 = run_bass_kernel_spmd(nc2, in_maps2, cores).results

    out = np.zeros((N, 2), np.float32)
    for c in cores:
        no = plan["cores"][c]["node_order"].reshape(n_pb, P)
        valid = no >= 0
        o = np.asarray(r2[c]["TOUT2"]).reshape(P, n_pb, 2).transpose(1, 0, 2)
        out[no[valid]] = o[valid]
    return out


def kernel(**inputs):
    out = run_gat(
        x=np.asarray(inputs["x"], np.float32),
        W1=np.asarray(inputs["W1"], np.float32),
        a1_src=np.asarray(inputs["a1_src"], np.float32),
        a1_dst=np.asarray(inputs["a1_dst"], np.float32),
        b1=np.asarray(inputs["b1"], np.float32),
        W2=np.asarray(inputs["W2"], np.float32),
        a2_src=np.asarray(inputs["a2_src"], np.float32),
        a2_dst=np.asarray(inputs["a2_dst"], np.float32),
        b2=np.asarray(inputs["b2"], np.float32),
        edge_index=np.asarray(inputs["edge_index"]),
    )
    return out.astype(np.float32)


# revision 73
# speedup vs baseline: 1.0296x; 1.0090x over previous
"""Two-layer GAT (GATConv x2 + log_softmax) on 8 Trainium2 NeuronCores.

Strategy (edge-parallel by dst node, zero on-device gathers):
  Host does pure index relabeling (argsort / np.take / dtype casts); all
  FLOPs run on-device across 3 SPMD launches.

  L0: nodes-on-partitions. One bf16 matmul per 128-node tile against
      Wcat = [W1 | W1@a_src | W1@a_dst] (built on device) yields h1 and
      both attention scores s_src/s_dst in a single pass.
  host: edges sorted by dst; per-node degree-bucketed padded slot layout
      (granularity 8), buckets dealt round-robin across the 8 cores so
      every core gets an identical block layout; per-slot streams:
      s_src (1 bf16) + h1 rows (8 bf16), dummy slots get s_src = -100
      so their softmax weight underflows to ~0.
  L1: e = s_src + s_dst (DVE/Pool), w = Exp(Prelu(e, 0.2)) on the ACT
      engine written straight into a 9th feature row of the q tile, so
      the z = sum(w) rides the same halving add-tree as the weighted
      feature sums. mults/trees split across DVE+GPSIMD by a per-chunk
      schedule; out1 = agg/(z+1e-16)+b1, relu, h2 = @W2, s2 scores.
  host: same relabeling for layer-2 streams (h2 + s2 scores).
  L2: same reduction with 2 features + w row, then log_softmax (f32).
"""

import numpy as np
import ml_dtypes

import concourse.bass as bass
import concourse.bacc as bacc
import concourse.mybir as mybir
import concourse.tile as tile
from concourse.bass_utils import run_bass_kernel_spmd

bf16 = ml_dtypes.bfloat16
P = 128
N_CORES = 8
GRAN = 8
SENTINEL = -100.0
F32 = mybir.dt.float32
BF16 = mybir.dt.bfloat16
AF = mybir.ActivationFunctionType
ALU = mybir.AluOpType
AX = mybir.AxisListType


# ----------------------------------------------------------------------
# Host-side plan (index arithmetic only)
# ----------------------------------------------------------------------

def build_plan(dst, n_nodes):
    E = dst.shape[0]
    order = np.argsort(dst, kind="stable").astype(np.int64)
    deg = np.bincount(dst, minlength=n_nodes).astype(np.int64)
    starts = np.zeros(n_nodes + 1, np.int64)
    np.cumsum(deg, out=starts[1:])

    Dpad = np.maximum(((deg + GRAN - 1) // GRAN) * GRAN, GRAN)
    Ds = np.unique(Dpad)

    # deal each bucket's nodes round-robin across cores: per-core counts are
    # equal (+-1) so the unified block counts carry no cross-core padding
    nblk = {}
    percore_nodes = [dict() for _ in range(N_CORES)]
    for D in Ds:
        sel = np.nonzero(Dpad == D)[0]
        for c in range(N_CORES):
            mine = sel[c::N_CORES]
            percore_nodes[c][int(D)] = mine
            nb = (len(mine) + P - 1) // P
            nblk[int(D)] = max(nblk.get(int(D), 0), nb)
    buckets = [(int(D), nblk[int(D)]) for D in Ds if nblk[int(D)] > 0][::-1]
    bounds = None

    n_pb = sum(nb for _, nb in buckets)             # node-blocks per partition
    S_part = sum(D * nb for D, nb in buckets)       # slots per partition

    cores = []
    for c in range(N_CORES):
        node_order = []
        for D, nb in buckets:
            sel = percore_nodes[c][D]
            padded = np.full(nb * P, -1, np.int64)
            padded[: len(sel)] = sel
            node_order.append(padded)
        node_order = np.concatenate(node_order)      # [(bucket,block,partition)]
        # perm: [P, S_part] edge index or -1, laid out per partition as
        # concat over buckets of [nblk, D]
        perm = np.full((P, S_part), -1, np.int64)
        no = node_order.reshape(n_pb, P)
        off = 0
        bi = 0
        for D, nb in buckets:
            for j in range(nb):
                nid = no[bi + j]                      # [P]
                for p in range(P):
                    n = nid[p]
                    if n < 0:
                        continue
                    s0 = starts[n]
                    cdeg = deg[n]
                    perm[p, off + j * D : off + j * D + cdeg] = order[s0 : s0 + cdeg]
            off += nb * D
            bi += nb
        cores.append({"node_order": node_order, "perm": perm})

    return {
        "buckets": buckets,
        "n_pb": n_pb,
        "S_part": S_part,
        "bounds": bounds,
        "cores": cores,
    }


def build_streams(plan, core_idx, src, s_tab, H_tab):
    """s_tab [N] f32 (src attention score), H_tab [N,F] bf16. Returns
    SA bf16 [P, S_part] flat s_src per slot (dummy slots = SENTINEL so
    their softmax weight underflows to ~0) and SH bf16
    [P, sum F*D*nblk] with per-bucket layout [nblk, F, D] per part."""
    core = plan["cores"][core_idx]
    perm = core["perm"]
    valid = perm >= 0
    Fh = H_tab.shape[1]
    src_slot = np.where(valid, src[np.clip(perm, 0, None)], 0)
    Aslot = s_tab[src_slot].astype(np.float32)   # [P, S]
    Aslot[~valid] = SENTINEL
    Hslot = H_tab[src_slot]                      # [P, S, F]
    Hslot[~valid] = 0
    parts = []
    off = 0
    for D, nb in plan["buckets"]:
        n = nb * D
        h = Hslot[:, off : off + n].reshape(P, nb, D, Fh).transpose(0, 1, 3, 2)
        parts.append(np.ascontiguousarray(h).reshape(P, -1).astype(bf16))
        off += n
    return Aslot.astype(bf16), np.concatenate(parts, axis=1)


def build_tbe(plan, core_idx, tab):
    """tab [N] f32 -> [P, S_part] bf16: per-slot s_dst (node value repeated
    across its D slots; zeros for dummy nodes)."""
    core = plan["cores"][core_idx]
    no = core["node_order"].reshape(plan["n_pb"], P)
    valid = no >= 0
    t = tab[np.clip(no, 0, None)].copy()         # [n_pb, P]
    t[~valid] = 0.0
    parts = []
    nbo = 0
    for D, nb in plan["buckets"]:
        blk = t[nbo : nbo + nb].T                # [P, nb]
        parts.append(np.repeat(blk[:, :, None], D, axis=2).reshape(P, nb * D))
        nbo += nb
    return np.concatenate(parts, axis=1).astype(bf16)


def build_node_tab(plan, core_idx, tab, k):
    """tab [N,k] f32 -> [P, n_pb, k] per layout order (zeros for dummies)."""
    core = plan["cores"][core_idx]
    no = core["node_order"].reshape(plan["n_pb"], P)
    valid = no >= 0
    t = tab[np.clip(no, 0, None)].copy()         # [n_pb, P, k]
    t[~valid] = 0.0
    return np.ascontiguousarray(t.transpose(1, 0, 2)).astype(bf16)


# ----------------------------------------------------------------------
# Launch 0: h1 = x@W1, s_src/s_dst scores - nodes on partitions
# ----------------------------------------------------------------------
NC_NODES = 12500
NC_PAD = 12544          # 98 tiles of 128
NT0 = NC_PAD // P
GB0 = 49                # tiles per PSUM group (49*10*4B = 1960B < 2KB bank)


def build_l0():
    nc = bacc.Bacc(None)
    xT = nc.dram_tensor("xT", [36, NC_PAD], BF16, kind="ExternalInput")
    W1 = nc.dram_tensor("W1", [36, 8], F32, kind="ExternalInput")
    W1TA = nc.dram_tensor("W1TA", [8, 38], F32, kind="ExternalInput")  # [W1T|a1s|a1d]
    HOUT = nc.dram_tensor("HOUT", [P, NT0 * 8], BF16, kind="ExternalOutput")
    SOUT = nc.dram_tensor("SOUT", [P, NT0 * 2], F32, kind="ExternalOutput")

    with tile.TileContext(nc) as tc:
        with (
            tc.tile_pool(name="cst", bufs=1) as cst,
            tc.tile_pool(name="sb", bufs=2) as sb,
            tc.tile_pool(name="ps", bufs=2, space="PSUM") as ps,
            tc.tile_pool(name="big", bufs=2) as big,
        ):
            w1 = cst.tile([36, 8], F32)
            w1ta = cst.tile([8, 38], F32)
            nc.sync.dma_start(out=w1[:], in_=W1[:])
            nc.sync.dma_start(out=w1ta[:], in_=W1TA[:])
            wsd_ps = ps.tile([36, 2], F32, tag="wsd")
            nc.tensor.matmul(wsd_ps[:], lhsT=w1ta[:, 0:36], rhs=w1ta[:, 36:38],
                             start=True, stop=True)
            wcat = cst.tile([36, 10], BF16)
            nc.vector.tensor_copy(out=wcat[:, 0:8], in_=w1[:])
            nc.vector.tensor_copy(out=wcat[:, 8:10], in_=wsd_ps[:])

            for g in range(0, NT0, GB0):
                gn = min(GB0, NT0 - g)
                xt = sb.tile([36, GB0 * P], BF16, tag="xt")
                nc.sync.dma_start(out=xt[:, : gn * P],
                                  in_=xT[:, g * P : (g + gn) * P])
                pst = ps.tile([P, GB0, 10], F32, tag="pst")
                for k in range(gn):
                    nc.tensor.matmul(pst[:, k, :],
                                     lhsT=xt[:, k * P : (k + 1) * P],
                                     rhs=wcat[:], start=True, stop=True)
                hs = big.tile([P, GB0, 8], BF16, tag="hs")
                ss = big.tile([P, GB0, 2], F32, tag="ss")
                nc.vector.tensor_copy(out=hs[:, :gn, :], in_=pst[:, :gn, 0:8])
                nc.vector.tensor_copy(out=ss[:, :gn, :], in_=pst[:, :gn, 8:10])
                nc.sync.dma_start(
                    out=HOUT[:, g * 8 : (g + gn) * 8],
                    in_=hs[:, :gn].rearrange("p a b -> p (a b)"))
                nc.sync.dma_start(
                    out=SOUT[:, g * 2 : (g + gn) * 2],
                    in_=ss[:, :gn].rearrange("p a b -> p (a b)"))
    nc.finalize()
    return nc


# ----------------------------------------------------------------------
# Launch 1 / Launch 2 shared reduction kernel
# ----------------------------------------------------------------------
CHUNK1 = 1792           # slots per partition per chunk (layer 1)
CHUNK2 = 1536           # slots per partition per chunk (layer 2)


def build_reduce_layer(plan, Fh, layer):
    """layer==1: h1 agg + out1 + h2/s2 tail. layer==2: h2 agg + log_softmax.
    One merged stream SH [P, sum (Fh+1)*D*nb]: rows 0:Fh features, row Fh
    s_src. TB [P, n_pb] s_dst. Row Fh is rewritten in place to
    w = exp(leaky_relu(s_src+s_dst, 0.2)) so z folds into the add-tree."""
    buckets = plan["buckets"]
    n_pb = plan["n_pb"]
    S_part = plan["S_part"]
    FT = Fh + 1
    LH = sum(Fh * D * nb for D, nb in buckets)
    chunk = CHUNK1 if layer == 1 else CHUNK2

    nc = bacc.Bacc(None)
    SA = nc.dram_tensor("SA", [P, S_part], BF16, kind="ExternalInput")
    SH = nc.dram_tensor("SH", [P, LH], BF16, kind="ExternalInput")
    TB = nc.dram_tensor("TB", [P, n_pb], BF16, kind="ExternalInput")
    BIAS = nc.dram_tensor("BIAS", [P, Fh], F32, kind="ExternalInput")
    if layer == 1:
        W2R = nc.dram_tensor("W2R", [P, 2 * 8], F32, kind="ExternalInput")
        A2V = nc.dram_tensor("A2V", [P, 2 * 2], F32, kind="ExternalInput")
        TOUTH = nc.dram_tensor("TOUTH", [P, n_pb * 2], BF16, kind="ExternalOutput")
        TOUTS = nc.dram_tensor("TOUTS", [P, n_pb * 2], F32, kind="ExternalOutput")
    else:
        TOUT2 = nc.dram_tensor("TOUT2", [P, n_pb * 2], F32, kind="ExternalOutput")

    with tile.TileContext(nc) as tc:
        with (
            tc.tile_pool(name="cst", bufs=1) as cst,
            tc.tile_pool(name="acc", bufs=1) as accp,
            tc.tile_pool(name="sa", bufs=4 if layer == 1 else 8) as sap,
            tc.tile_pool(name="wp", bufs=6) as wpp,
            tc.tile_pool(name="sh", bufs=4 if layer == 1 else 8) as shp,
        ):
            tb = cst.tile([P, n_pb, 1], BF16)
            nc.gpsimd.dma_start(out=tb[:, :, 0], in_=TB[:])
            bias = cst.tile([P, 1, Fh], F32)
            gacc = accp.tile([P, n_pb, FT], F32)

            # tail tiles + emitter: the first half runs while later buckets
            # are still reducing, hiding most of the serial epilogue
            rz = accp.tile([P, n_pb, 1], F32)
            out = accp.tile([P, n_pb, Fh], F32)
            if layer == 1:
                w2r = cst.tile([P, 2, 8], F32)
                a2v = cst.tile([P, 2, 2], F32)
                h2in = accp.tile([P, n_pb, 8], F32)
                h2 = accp.tile([P, n_pb, 2], F32)
                tmp = accp.tile([P, n_pb, 8], F32)
                tmpg = accp.tile([P, n_pb, 8], F32)
                s2 = accp.tile([P, n_pb, 2], F32)
                tmp2 = accp.tile([P, n_pb, 2], F32)
                outh = accp.tile([P, n_pb, 2], BF16)
            else:
                tm = accp.tile([P, n_pb, 1], F32)
                tt2 = accp.tile([P, n_pb, 2], F32)
                te = accp.tile([P, n_pb, 2], F32)
                tss = accp.tile([P, n_pb, 1], F32)
                tls = accp.tile([P, n_pb, 1], F32)
                res = accp.tile([P, n_pb, 2], F32)
            tail_upto = [0]

            def emit_tail(b0, b1):
                if b0 >= b1:
                    return
                n = b1 - b0
                s = slice(b0, b1)
                if b0 == 0:
                    nc.sync.dma_start(out=bias[:, 0, :], in_=BIAS[:])
                    if layer == 1:
                        nc.sync.dma_start(
                            out=w2r[:],
                            in_=W2R[:].rearrange("p (a b) -> p a b", a=2))
                        nc.sync.dma_start(
                            out=a2v[:],
                            in_=A2V[:].rearrange("p (a b) -> p a b", a=2))
                nc.vector.tensor_scalar_add(out=rz[:, s, 0], in0=gacc[:, s, Fh],
                                            scalar1=1e-16)
                nc.vector.reciprocal(out=rz[:, s, 0], in_=rz[:, s, 0])
                nc.vector.tensor_tensor(
                    out=out[:, s], in0=gacc[:, s, 0:Fh],
                    in1=rz[:, s].to_broadcast([P, n, Fh]), op=ALU.mult)
                nc.vector.tensor_tensor(
                    out=out[:, s], in0=out[:, s],
                    in1=bias[:].to_broadcast([P, n, Fh]), op=ALU.add)
                if layer == 1:
                    nc.vector.tensor_relu(out=h2in[:, s], in_=out[:, s])
                    for c, eng, tt in ((0, nc.vector, tmp), (1, nc.gpsimd, tmpg)):
                        eng.tensor_tensor(
                            out=tt[:, s], in0=h2in[:, s],
                            in1=w2r[:, c : c + 1, :].to_broadcast([P, n, 8]),
                            op=ALU.mult)
                        nc.vector.tensor_reduce(out=h2[:, s, c : c + 1],
                                                in_=tt[:, s], axis=AX.X, op=ALU.add)
                    for c in range(2):
                        nc.vector.tensor_tensor(
                            out=tmp2[:, s], in0=h2[:, s],
                            in1=a2v[:, c : c + 1, :].to_broadcast([P, n, 2]),
                            op=ALU.mult)
                        nc.vector.tensor_reduce(out=s2[:, s, c : c + 1],
                                                in_=tmp2[:, s], axis=AX.X, op=ALU.add)
                    nc.vector.tensor_copy(out=outh[:, s], in_=h2[:, s])
                    nc.sync.dma_start(
                        out=TOUTH[:, b0 * 2 : b1 * 2],
                        in_=outh[:, s].rearrange("p a b -> p (a b)"))
                    nc.sync.dma_start(
                        out=TOUTS[:, b0 * 2 : b1 * 2],
                        in_=s2[:, s].rearrange("p a b -> p (a b)"))
                else:
                    nc.vector.tensor_tensor(out=tm[:, s], in0=out[:, s, 0:1],
                                            in1=out[:, s, 1:2], op=ALU.max)
                    nc.vector.tensor_tensor(
                        out=tt2[:, s], in0=out[:, s],
                        in1=tm[:, s].to_broadcast([P, n, 2]), op=ALU.subtract)
                    nc.scalar.activation(te[:, s], tt2[:, s], AF.Exp)
                    nc.vector.tensor_tensor(out=tss[:, s], in0=te[:, s, 0:1],
                                            in1=te[:, s, 1:2], op=ALU.add)
                    nc.scalar.activation(tls[:, s], tss[:, s], AF.Ln)
                    nc.vector.tensor_tensor(
                        out=res[:, s], in0=tt2[:, s],
                        in1=tls[:, s].to_broadcast([P, n, 2]), op=ALU.subtract)
                    nc.sync.dma_start(
                        out=TOUT2[:, b0 * 2 : b1 * 2],
                        in_=res[:, s].rearrange("p a b -> p (a b)"))

            def tree4(eng, t4, f0, f1, D, stop=8):
                d = D
                while d > stop:
                    h = d // 2
                    eng.tensor_tensor(
                        out=t4[:, :, f0:f1, 0:h], in0=t4[:, :, f0:f1, 0:h],
                        in1=t4[:, :, f0:f1, h : 2 * h], op=ALU.add)
                    if d % 2:
                        eng.tensor_tensor(
                            out=t4[:, :, f0:f1, 0:1], in0=t4[:, :, f0:f1, 0:1],
                            in1=t4[:, :, f0:f1, d - 1 : d], op=ALU.add)
                    d = h
                return d

            h_off = 0
            a_off = 0
            nb_off = 0
            for D, nb in buckets:
                n_ch = max(1, -(-(nb * D) // chunk))
                cb = -(-nb // n_ch)
                for j0 in range(0, nb, cb):
                    cbn = min(cb, nb - j0)
                    blks = slice(nb_off + j0, nb_off + j0 + cbn)
                    sa = sap.tile([P, cb, D], BF16, tag="sa")
                    nc.sync.dma_start(
                        out=sa[:, :cbn],
                        in_=SA[:, a_off + j0 * D : a_off + (j0 + cbn) * D]
                        .rearrange("p (c d) -> p c d", d=D))
                    if layer == 1:
                        # w lives as feature row Fh of q: z rides the q tree
                        q = shp.tile([P, cb, FT, D], BF16, tag="q")
                        nc.sync.dma_start(
                            out=q[:, :cbn, 0:Fh, :],
                            in_=SH[:, h_off + j0 * Fh * D : h_off + (j0 + cbn) * Fh * D]
                            .rearrange("p (c k d) -> p c k d", k=Fh, d=D))
                        wv = q[:, :cbn, Fh:FT, :]
                    else:
                        # separate w tile keeps the SH DMA fully contiguous
                        q = shp.tile([P, cb, Fh, D], BF16, tag="q")
                        nc.sync.dma_start(
                            out=q[:, :cbn],
                            in_=SH[:, h_off + j0 * Fh * D : h_off + (j0 + cbn) * Fh * D]
                            .rearrange("p (c k d) -> p c k d", k=Fh, d=D))
                        w = wpp.tile([P, cb, 1, D], BF16, tag="w")
                        wv = w[:, :cbn]
                    # e = s_src + s_dst (Pool); w = exp(leaky_relu(e, .2)) (ACT)
                    nc.gpsimd.tensor_tensor(
                        out=sa[:, :cbn, :], in0=sa[:, :cbn, :],
                        in1=tb[:, blks, :].to_broadcast([P, cbn, D]),
                        op=ALU.add)
                    nc.scalar.activation(sa[:, :cbn, :], sa[:, :cbn, :],
                                         AF.Prelu, alpha=0.2)
                    nc.scalar.activation(wv[:, :, 0, :], sa[:, :cbn, :],
                                         AF.Exp)

                    if layer == 1:
                        # mults: DVE f0:7, Pool f7; trees: DVE f0:7, Pool f7:9
                        nc.vector.tensor_tensor(
                            out=q[:, :cbn, 0:7, :], in0=q[:, :cbn, 0:7, :],
                            in1=wv.to_broadcast([P, cbn, 7, D]), op=ALU.mult)
                        nc.gpsimd.tensor_tensor(
                            out=q[:, :cbn, 7:8, :], in0=q[:, :cbn, 7:8, :],
                            in1=wv.to_broadcast([P, cbn, 1, D]), op=ALU.mult)
                        d = tree4(nc.vector, q[:, :cbn], 0, 7, D)
                        tree4(nc.gpsimd, q[:, :cbn], 7, FT, D)
                        nc.vector.tensor_reduce(
                            out=gacc[:, blks, :],
                            in_=q[:, :cbn, :, 0:d], axis=AX.X, op=ALU.add)
                    else:
                        # mults/feature-trees on DVE; w(z)-tree on Pool
                        nc.vector.tensor_tensor(
                            out=q[:, :cbn, :, :], in0=q[:, :cbn, :, :],
                            in1=wv.to_broadcast([P, cbn, Fh, D]), op=ALU.mult)
                        d = tree4(nc.vector, q[:, :cbn], 0, Fh, D)
                        dw = tree4(nc.vector, w[:, :cbn], 0, 1, D)
                        nc.vector.tensor_reduce(
                            out=gacc[:, blks, 0:Fh],
                            in_=q[:, :cbn, :, 0:d], axis=AX.X, op=ALU.add)
                        nc.vector.tensor_reduce(
                            out=gacc[:, blks, Fh:FT],
                            in_=w[:, :cbn, :, 0:dw], axis=AX.X, op=ALU.add)
                h_off += nb * Fh * D
                a_off += nb * D
                nb_off += nb

            emit_tail(tail_upto[0], n_pb)
    nc.finalize()
    return nc


# ----------------------------------------------------------------------
# Orchestration
# ----------------------------------------------------------------------

def run_gat(x, W1, a1_src, a1_dst, b1, W2, a2_src, a2_dst, b2, edge_index):
    N = x.shape[0]
    src = np.asarray(edge_index[0], np.int64)
    dst = np.asarray(edge_index[1], np.int64)
    plan = build_plan(dst, N)
    n_pb = plan["n_pb"]
    cores = list(range(N_CORES))

    # ---------------- L0 ----------------
    nc0 = build_l0()
    xpad = np.zeros((N_CORES, NC_PAD, 36), bf16)
    xpad[:, :NC_NODES] = np.asarray(x).reshape(N_CORES, NC_NODES, 36).astype(bf16)
    W1f = np.asarray(W1, np.float32)
    AVf = np.stack([np.asarray(a1_src, np.float32), np.asarray(a1_dst, np.float32)], 1)
    in_maps0 = []
    W1TA = np.ascontiguousarray(np.concatenate([W1f.T, AVf], axis=1))
    for c in cores:
        in_maps0.append({
            "xT": np.ascontiguousarray(xpad[c].T),
            "W1": W1f, "W1TA": W1TA,
        })
    r0 = run_bass_kernel_spmd(nc0, in_maps0, cores).results
    h1bf = np.zeros((N, 8), bf16)
    s1s = np.zeros((N,), np.float32)
    s1d = np.zeros((N,), np.float32)
    for c in cores:
        # node t*P+p lives at [p, t, :]
        hcore = np.asarray(r0[c]["HOUT"]).reshape(P, NT0, 8).transpose(1, 0, 2)
        score = np.asarray(r0[c]["SOUT"]).reshape(P, NT0, 2).transpose(1, 0, 2)
        sl = slice(c * NC_NODES, (c + 1) * NC_NODES)
        h1bf[sl] = hcore.reshape(NC_PAD, 8)[:NC_NODES]
        s = score.reshape(NC_PAD, 2)[:NC_NODES]
        s1s[sl] = s[:, 0]
        s1d[sl] = s[:, 1]

    # ---------------- L1 ----------------
    nc1 = build_reduce_layer(plan, 8, 1)
    b1rep = np.tile(np.asarray(b1, np.float32)[None, :], (P, 1))
    W2rep = np.tile(np.asarray(W2, np.float32).T.reshape(1, 2, 8), (P, 1, 1)).reshape(P, 16)
    A2Vrep = np.tile(np.stack([np.asarray(a2_src, np.float32), np.asarray(a2_dst, np.float32)], 0).reshape(1, 2, 2), (P, 1, 1)).reshape(P, 4)
    in_maps1 = []
    for c in cores:
        SAc, SHc = build_streams(plan, c, src, s1s, h1bf)
        TB = build_node_tab(plan, c, s1d[:, None], 1).reshape(P, -1)
        in_maps1.append({
            "SA": SAc, "SH": SHc, "TB": TB, "BIAS": b1rep,
            "W2R": W2rep, "A2V": A2Vrep,
        })
    r1 = run_bass_kernel_spmd(nc1, in_maps1, cores).results

    # host: rebuild global tables for L2
    h2bf = np.zeros((N, 2), bf16)
    s2s = np.zeros((N,), np.float32)
    s2d = np.zeros((N,), np.float32)
    for c in cores:
        no = plan["cores"][c]["node_order"].reshape(n_pb, P)
        valid = no >= 0
        outh = np.asarray(r1[c]["TOUTH"]).reshape(P, n_pb, 2).transpose(1, 0, 2)
        outs = np.asarray(r1[c]["TOUTS"]).reshape(P, n_pb, 2).transpose(1, 0, 2)
        h2bf[no[valid]] = outh[valid]
        s2s[no[valid]] = outs[valid][:, 0]
        s2d[no[valid]] = outs[valid][:, 1]

    # ---------------- L2 ----------------
    nc2 = build_reduce_layer(plan, 2, 2)
    b2rep = np.tile(np.asarray(b2, np.float32)[None, :], (P, 1))
    in_maps2 = []
    for c in cores:
        SAc, SHc = build_streams(plan, c, src, s2s, h2bf)
        TB = build_node_tab(plan, c, s2d[:, None], 1).reshape(P, -1)
        in_maps2.append({"SA": SAc, "SH": SHc, "TB": TB, "BIAS": b2rep})
    r2 = run_bass_kernel_spmd(nc2, in_maps2, cores).results

    out = np.zeros((N, 2), np.float32)
    for c in cores:
        no = plan["cores"][c]["node_order"].reshape(n_pb, P)
        valid = no >= 0
        o = np.asarray(r2[c]["TOUT2"]).reshape(P, n_pb, 2).transpose(1, 0, 2)
        out[no[valid]] = o[valid]
    return out


def kernel(**inputs):
    out = run_gat(
        x=np.asarray(inputs["x"], np.float32),
        W1=np.asarray(inputs["W1"], np.float32),
        a1_src=np.asarray(inputs["a1_src"], np.float32),
        a1_dst=np.asarray(inputs["a1_dst"], np.float32),
        b1=np.asarray(inputs["b1"], np.float32),
        W2=np.asarray(inputs["W2"], np.float32),
        a2_src=np.asarray(inputs["a2_src"], np.float32),
        a2_dst=np.asarray(inputs["a2_dst"], np.float32),
        b2=np.asarray(inputs["b2"], np.float32),
        edge_index=np.asarray(inputs["edge_index"]),
    )
    return out.astype(np.float32)


# revision 74
# speedup vs baseline: 1.0509x; 1.0207x over previous
"""Two-layer GAT (GATConv x2 + log_softmax) on 8 Trainium2 NeuronCores.

Strategy (edge-parallel by dst node, zero on-device gathers):
  Host does pure index relabeling (argsort / np.take / dtype casts); all
  FLOPs run on-device across 3 SPMD launches.

  L0: nodes-on-partitions. One bf16 matmul per 128-node tile against
      Wcat = [W1 | W1@a_src | W1@a_dst] (built on device) yields h1 and
      both attention scores s_src/s_dst in a single pass.
  host: edges sorted by dst; per-node degree-bucketed padded slot layout
      (granularity 8), buckets dealt round-robin across the 8 cores so
      every core gets an identical block layout; per-slot streams:
      s_src (1 bf16) + h1 rows (8 bf16), dummy slots get s_src = -100
      so their softmax weight underflows to ~0.
  L1: e = s_src + s_dst (DVE/Pool), w = Exp(Prelu(e, 0.2)) on the ACT
      engine written straight into a 9th feature row of the q tile, so
      the z = sum(w) rides the same halving add-tree as the weighted
      feature sums. mults/trees split across DVE+GPSIMD by a per-chunk
      schedule; out1 = agg/(z+1e-16)+b1, relu, h2 = @W2, s2 scores.
  host: same relabeling for layer-2 streams (h2 + s2 scores).
  L2: same reduction with 2 features + w row, then log_softmax (f32).
"""

import numpy as np
import ml_dtypes

import concourse.bass as bass
import concourse.bacc as bacc
import concourse.mybir as mybir
import concourse.tile as tile
from concourse.bass_utils import run_bass_kernel_spmd

bf16 = ml_dtypes.bfloat16
P = 128
N_CORES = 8
GRAN = 8
SENTINEL = -100.0
F32 = mybir.dt.float32
BF16 = mybir.dt.bfloat16
AF = mybir.ActivationFunctionType
ALU = mybir.AluOpType
AX = mybir.AxisListType


# ----------------------------------------------------------------------
# Host-side plan (index arithmetic only)
# ----------------------------------------------------------------------

def build_plan(dst, n_nodes):
    E = dst.shape[0]
    order = np.argsort(dst, kind="stable").astype(np.int64)
    deg = np.bincount(dst, minlength=n_nodes).astype(np.int64)
    starts = np.zeros(n_nodes + 1, np.int64)
    np.cumsum(deg, out=starts[1:])

    Dpad = np.maximum(((deg + GRAN - 1) // GRAN) * GRAN, GRAN)
    Ds = np.unique(Dpad)

    # deal each bucket's nodes round-robin across cores: per-core counts are
    # equal (+-1) so the unified block counts carry no cross-core padding
    nblk = {}
    percore_nodes = [dict() for _ in range(N_CORES)]
    for D in Ds:
        sel = np.nonzero(Dpad == D)[0]
        for c in range(N_CORES):
            mine = sel[c::N_CORES]
            percore_nodes[c][int(D)] = mine
            nb = (len(mine) + P - 1) // P
            nblk[int(D)] = max(nblk.get(int(D), 0), nb)
    buckets = [(int(D), nblk[int(D)]) for D in Ds if nblk[int(D)] > 0][::-1]
    bounds = None

    n_pb = sum(nb for _, nb in buckets)             # node-blocks per partition
    S_part = sum(D * nb for D, nb in buckets)       # slots per partition

    cores = []
    for c in range(N_CORES):
        node_order = []
        for D, nb in buckets:
            sel = percore_nodes[c][D]
            padded = np.full(nb * P, -1, np.int64)
            padded[: len(sel)] = sel
            node_order.append(padded)
        node_order = np.concatenate(node_order)      # [(bucket,block,partition)]
        # perm: [P, S_part] edge index or -1, laid out per partition as
        # concat over buckets of [nblk, D]
        perm = np.full((P, S_part), -1, np.int64)
        no = node_order.reshape(n_pb, P)
        off = 0
        bi = 0
        for D, nb in buckets:
            for j in range(nb):
                nid = no[bi + j]                      # [P]
                for p in range(P):
                    n = nid[p]
                    if n < 0:
                        continue
                    s0 = starts[n]
                    cdeg = deg[n]
                    perm[p, off + j * D : off + j * D + cdeg] = order[s0 : s0 + cdeg]
            off += nb * D
            bi += nb
        cores.append({"node_order": node_order, "perm": perm})

    return {
        "buckets": buckets,
        "n_pb": n_pb,
        "S_part": S_part,
        "bounds": bounds,
        "cores": cores,
    }


def build_streams(plan, core_idx, src, s_tab, H_tab):
    """s_tab [N] f32 (src attention score), H_tab [N,F] bf16. Returns
    SA bf16 [P, S_part] flat s_src per slot (dummy slots = SENTINEL so
    their softmax weight underflows to ~0) and SH bf16
    [P, sum F*D*nblk] with per-bucket layout [nblk, F, D] per part."""
    core = plan["cores"][core_idx]
    perm = core["perm"]
    valid = perm >= 0
    Fh = H_tab.shape[1]
    src_slot = np.where(valid, src[np.clip(perm, 0, None)], 0)
    Aslot = s_tab[src_slot].astype(np.float32)   # [P, S]
    Aslot[~valid] = SENTINEL
    Hslot = H_tab[src_slot]                      # [P, S, F]
    Hslot[~valid] = 0
    parts = []
    off = 0
    for D, nb in plan["buckets"]:
        n = nb * D
        h = Hslot[:, off : off + n].reshape(P, nb, D, Fh).transpose(0, 1, 3, 2)
        parts.append(np.ascontiguousarray(h).reshape(P, -1).astype(bf16))
        off += n
    return Aslot.astype(bf16), np.concatenate(parts, axis=1)


def build_tbe(plan, core_idx, tab):
    """tab [N] f32 -> [P, S_part] bf16: per-slot s_dst (node value repeated
    across its D slots; zeros for dummy nodes)."""
    core = plan["cores"][core_idx]
    no = core["node_order"].reshape(plan["n_pb"], P)
    valid = no >= 0
    t = tab[np.clip(no, 0, None)].copy()         # [n_pb, P]
    t[~valid] = 0.0
    parts = []
    nbo = 0
    for D, nb in plan["buckets"]:
        blk = t[nbo : nbo + nb].T                # [P, nb]
        parts.append(np.repeat(blk[:, :, None], D, axis=2).reshape(P, nb * D))
        nbo += nb
    return np.concatenate(parts, axis=1).astype(bf16)


def build_node_tab(plan, core_idx, tab, k):
    """tab [N,k] f32 -> [P, n_pb, k] per layout order (zeros for dummies)."""
    core = plan["cores"][core_idx]
    no = core["node_order"].reshape(plan["n_pb"], P)
    valid = no >= 0
    t = tab[np.clip(no, 0, None)].copy()         # [n_pb, P, k]
    t[~valid] = 0.0
    return np.ascontiguousarray(t.transpose(1, 0, 2)).astype(bf16)


# ----------------------------------------------------------------------
# Launch 0: h1 = x@W1, s_src/s_dst scores - nodes on partitions
# ----------------------------------------------------------------------
NC_NODES = 12500
NC_PAD = 12544          # 98 tiles of 128
NT0 = NC_PAD // P
GB0 = 49                # tiles per PSUM group (49*10*4B = 1960B < 2KB bank)


def build_l0():
    nc = bacc.Bacc(None)
    xT = nc.dram_tensor("xT", [36, NC_PAD], BF16, kind="ExternalInput")
    W1 = nc.dram_tensor("W1", [36, 8], F32, kind="ExternalInput")
    W1TA = nc.dram_tensor("W1TA", [8, 38], F32, kind="ExternalInput")  # [W1T|a1s|a1d]
    HOUT = nc.dram_tensor("HOUT", [P, NT0 * 8], BF16, kind="ExternalOutput")
    SOUT = nc.dram_tensor("SOUT", [P, NT0 * 2], F32, kind="ExternalOutput")

    with tile.TileContext(nc) as tc:
        with (
            tc.tile_pool(name="cst", bufs=1) as cst,
            tc.tile_pool(name="sb", bufs=2) as sb,
            tc.tile_pool(name="ps", bufs=2, space="PSUM") as ps,
            tc.tile_pool(name="big", bufs=2) as big,
        ):
            w1 = cst.tile([36, 8], F32)
            w1ta = cst.tile([8, 38], F32)
            nc.sync.dma_start(out=w1[:], in_=W1[:])
            nc.sync.dma_start(out=w1ta[:], in_=W1TA[:])
            wsd_ps = ps.tile([36, 2], F32, tag="wsd")
            nc.tensor.matmul(wsd_ps[:], lhsT=w1ta[:, 0:36], rhs=w1ta[:, 36:38],
                             start=True, stop=True)
            wcat = cst.tile([36, 10], BF16)
            nc.vector.tensor_copy(out=wcat[:, 0:8], in_=w1[:])
            nc.vector.tensor_copy(out=wcat[:, 8:10], in_=wsd_ps[:])

            for g in range(0, NT0, GB0):
                gn = min(GB0, NT0 - g)
                xt = sb.tile([36, GB0 * P], BF16, tag="xt")
                nc.sync.dma_start(out=xt[:, : gn * P],
                                  in_=xT[:, g * P : (g + gn) * P])
                pst = ps.tile([P, GB0, 10], F32, tag="pst")
                for k in range(gn):
                    nc.tensor.matmul(pst[:, k, :],
                                     lhsT=xt[:, k * P : (k + 1) * P],
                                     rhs=wcat[:], start=True, stop=True)
                hs = big.tile([P, GB0, 8], BF16, tag="hs")
                ss = big.tile([P, GB0, 2], F32, tag="ss")
                nc.vector.tensor_copy(out=hs[:, :gn, :], in_=pst[:, :gn, 0:8])
                nc.vector.tensor_copy(out=ss[:, :gn, :], in_=pst[:, :gn, 8:10])
                nc.sync.dma_start(
                    out=HOUT[:, g * 8 : (g + gn) * 8],
                    in_=hs[:, :gn].rearrange("p a b -> p (a b)"))
                nc.sync.dma_start(
                    out=SOUT[:, g * 2 : (g + gn) * 2],
                    in_=ss[:, :gn].rearrange("p a b -> p (a b)"))
    nc.finalize()
    return nc


# ----------------------------------------------------------------------
# Launch 1 / Launch 2 shared reduction kernel
# ----------------------------------------------------------------------
CHUNK1 = 1792           # slots per partition per chunk (layer 1)
CHUNK2 = 1536           # slots per partition per chunk (layer 2)


def build_reduce_layer(plan, Fh, layer):
    """layer==1: h1 agg + out1 + h2/s2 tail. layer==2: h2 agg + log_softmax.
    One merged stream SH [P, sum (Fh+1)*D*nb]: rows 0:Fh features, row Fh
    s_src. TB [P, n_pb] s_dst. Row Fh is rewritten in place to
    w = exp(leaky_relu(s_src+s_dst, 0.2)) so z folds into the add-tree."""
    buckets = plan["buckets"]
    n_pb = plan["n_pb"]
    S_part = plan["S_part"]
    FT = Fh + 1
    LH = sum(Fh * D * nb for D, nb in buckets)
    chunk = CHUNK1 if layer == 1 else CHUNK2

    nc = bacc.Bacc(None)
    SA = nc.dram_tensor("SA", [P, S_part], BF16, kind="ExternalInput")
    SH = nc.dram_tensor("SH", [P, LH], BF16, kind="ExternalInput")
    TB = nc.dram_tensor("TB", [P, n_pb], BF16, kind="ExternalInput")
    BIAS = nc.dram_tensor("BIAS", [P, Fh], F32, kind="ExternalInput")
    if layer == 1:
        W2R = nc.dram_tensor("W2R", [P, 2 * 8], F32, kind="ExternalInput")
        A2V = nc.dram_tensor("A2V", [P, 2 * 2], F32, kind="ExternalInput")
        TOUTH = nc.dram_tensor("TOUTH", [P, n_pb * 2], BF16, kind="ExternalOutput")
        TOUTS = nc.dram_tensor("TOUTS", [P, n_pb * 2], F32, kind="ExternalOutput")
    else:
        TOUT2 = nc.dram_tensor("TOUT2", [P, n_pb * 2], F32, kind="ExternalOutput")

    with tile.TileContext(nc) as tc:
        with (
            tc.tile_pool(name="cst", bufs=1) as cst,
            tc.tile_pool(name="acc", bufs=1) as accp,
            tc.tile_pool(name="sa", bufs=4 if layer == 1 else 8) as sap,
            tc.tile_pool(name="wp", bufs=6) as wpp,
            tc.tile_pool(name="sh", bufs=4 if layer == 1 else 8) as shp,
        ):
            tb = cst.tile([P, n_pb, 1], BF16)
            nc.gpsimd.dma_start(out=tb[:, :, 0], in_=TB[:])
            bias = cst.tile([P, 1, Fh], F32)
            gacc = accp.tile([P, n_pb, FT], F32)

            # tail tiles + emitter: the first half runs while later buckets
            # are still reducing, hiding most of the serial epilogue
            rz = accp.tile([P, n_pb, 1], F32)
            out = accp.tile([P, n_pb, Fh], F32)
            if layer == 1:
                w2r = cst.tile([P, 2, 8], F32)
                a2v = cst.tile([P, 2, 2], F32)
                h2in = accp.tile([P, n_pb, 8], F32)
                h2 = accp.tile([P, n_pb, 2], F32)
                tmp = accp.tile([P, n_pb, 8], F32)
                tmpg = accp.tile([P, n_pb, 8], F32)
                s2 = accp.tile([P, n_pb, 2], F32)
                tmp2 = accp.tile([P, n_pb, 2], F32)
                outh = accp.tile([P, n_pb, 2], BF16)
            else:
                tm = accp.tile([P, n_pb, 1], F32)
                tt2 = accp.tile([P, n_pb, 2], F32)
                te = accp.tile([P, n_pb, 2], F32)
                tss = accp.tile([P, n_pb, 1], F32)
                tls = accp.tile([P, n_pb, 1], F32)
                res = accp.tile([P, n_pb, 2], F32)
            tail_upto = [0]

            def emit_tail(b0, b1):
                if b0 >= b1:
                    return
                n = b1 - b0
                s = slice(b0, b1)
                if b0 == 0:
                    nc.sync.dma_start(out=bias[:, 0, :], in_=BIAS[:])
                    if layer == 1:
                        nc.sync.dma_start(
                            out=w2r[:],
                            in_=W2R[:].rearrange("p (a b) -> p a b", a=2))
                        nc.sync.dma_start(
                            out=a2v[:],
                            in_=A2V[:].rearrange("p (a b) -> p a b", a=2))
                nc.vector.tensor_scalar_add(out=rz[:, s, 0], in0=gacc[:, s, Fh],
                                            scalar1=1e-16)
                nc.vector.reciprocal(out=rz[:, s, 0], in_=rz[:, s, 0])
                nc.vector.tensor_tensor(
                    out=out[:, s], in0=gacc[:, s, 0:Fh],
                    in1=rz[:, s].to_broadcast([P, n, Fh]), op=ALU.mult)
                nc.vector.tensor_tensor(
                    out=out[:, s], in0=out[:, s],
                    in1=bias[:].to_broadcast([P, n, Fh]), op=ALU.add)
                if layer == 1:
                    nc.vector.tensor_relu(out=h2in[:, s], in_=out[:, s])
                    for c, eng, tt in ((0, nc.vector, tmp), (1, nc.gpsimd, tmpg)):
                        eng.tensor_tensor(
                            out=tt[:, s], in0=h2in[:, s],
                            in1=w2r[:, c : c + 1, :].to_broadcast([P, n, 8]),
                            op=ALU.mult)
                        nc.vector.tensor_reduce(out=h2[:, s, c : c + 1],
                                                in_=tt[:, s], axis=AX.X, op=ALU.add)
                    for c in range(2):
                        nc.vector.tensor_tensor(
                            out=tmp2[:, s], in0=h2[:, s],
                            in1=a2v[:, c : c + 1, :].to_broadcast([P, n, 2]),
                            op=ALU.mult)
                        nc.vector.tensor_reduce(out=s2[:, s, c : c + 1],
                                                in_=tmp2[:, s], axis=AX.X, op=ALU.add)
                    nc.vector.tensor_copy(out=outh[:, s], in_=h2[:, s])
                    nc.sync.dma_start(
                        out=TOUTH[:, b0 * 2 : b1 * 2],
                        in_=outh[:, s].rearrange("p a b -> p (a b)"))
                    nc.sync.dma_start(
                        out=TOUTS[:, b0 * 2 : b1 * 2],
                        in_=s2[:, s].rearrange("p a b -> p (a b)"))
                else:
                    nc.vector.tensor_tensor(out=tm[:, s], in0=out[:, s, 0:1],
                                            in1=out[:, s, 1:2], op=ALU.max)
                    nc.vector.tensor_tensor(
                        out=tt2[:, s], in0=out[:, s],
                        in1=tm[:, s].to_broadcast([P, n, 2]), op=ALU.subtract)
                    nc.scalar.activation(te[:, s], tt2[:, s], AF.Exp)
                    nc.vector.tensor_tensor(out=tss[:, s], in0=te[:, s, 0:1],
                                            in1=te[:, s, 1:2], op=ALU.add)
                    nc.scalar.activation(tls[:, s], tss[:, s], AF.Ln)
                    nc.vector.tensor_tensor(
                        out=res[:, s], in0=tt2[:, s],
                        in1=tls[:, s].to_broadcast([P, n, 2]), op=ALU.subtract)
                    nc.sync.dma_start(
                        out=TOUT2[:, b0 * 2 : b1 * 2],
                        in_=res[:, s].rearrange("p a b -> p (a b)"))

            def tree4(eng, t4, f0, f1, D, stop=8):
                d = D
                while d > stop:
                    h = d // 2
                    eng.tensor_tensor(
                        out=t4[:, :, f0:f1, 0:h], in0=t4[:, :, f0:f1, 0:h],
                        in1=t4[:, :, f0:f1, h : 2 * h], op=ALU.add)
                    if d % 2:
                        eng.tensor_tensor(
                            out=t4[:, :, f0:f1, 0:1], in0=t4[:, :, f0:f1, 0:1],
                            in1=t4[:, :, f0:f1, d - 1 : d], op=ALU.add)
                    d = h
                return d

            # consecutive tiny buckets share one SA DMA + one SH DMA so the
            # schedule's head/tail don't pay per-bucket DMA latency
            SMALL = 272
            runs = []
            cur = []
            for D, nb in buckets:
                if nb * D <= SMALL:
                    cur.append((D, nb))
                else:
                    if cur:
                        runs.append(cur)
                        cur = []
                    runs.append([(D, nb)])
            if cur:
                runs.append(cur)

            h_off = 0
            a_off = 0
            nb_off = 0
            for run in runs:
                if run[0][0] * run[0][1] <= SMALL:
                    runS = sum(D * nb for D, nb in run)
                    saf = sap.tile([P, runS], BF16, tag="saf")
                    qf = shp.tile([P, runS * Fh], BF16, tag="qf")
                    nc.sync.dma_start(out=saf[:], in_=SA[:, a_off : a_off + runS])
                    nc.sync.dma_start(out=qf[:],
                                      in_=SH[:, h_off : h_off + runS * Fh])
                    ro = 0
                    for D, nb in run:
                        blks = slice(nb_off, nb_off + nb)
                        sav = saf[:, ro : ro + nb * D].rearrange(
                            "p (c d) -> p c d", d=D)
                        qv = qf[:, ro * Fh : (ro + nb * D) * Fh].rearrange(
                            "p (c k d) -> p c k d", k=Fh, d=D)
                        w = wpp.tile([P, nb, 1, D], BF16, tag="wsm")
                        nc.gpsimd.tensor_tensor(
                            out=sav, in0=sav,
                            in1=tb[:, blks, :].to_broadcast([P, nb, D]),
                            op=ALU.add)
                        nc.scalar.activation(sav, sav, AF.Prelu, alpha=0.2)
                        nc.scalar.activation(w[:, :, 0, :], sav, AF.Exp)
                        nc.vector.tensor_tensor(
                            out=qv, in0=qv,
                            in1=w[:].to_broadcast([P, nb, Fh, D]), op=ALU.mult)
                        d = tree4(nc.vector, qv, 0, Fh, D)
                        dw = tree4(nc.vector, w[:], 0, 1, D)
                        nc.vector.tensor_reduce(
                            out=gacc[:, blks, 0:Fh], in_=qv[:, :, :, 0:d],
                            axis=AX.X, op=ALU.add)
                        nc.vector.tensor_reduce(
                            out=gacc[:, blks, Fh:FT], in_=w[:, :, :, 0:dw],
                            axis=AX.X, op=ALU.add)
                        ro += nb * D
                        nb_off += nb
                        a_off += nb * D
                        h_off += nb * Fh * D
                    continue
                (D, nb), = run
                n_ch = max(1, -(-(nb * D) // chunk))
                cb = -(-nb // n_ch)
                for j0 in range(0, nb, cb):
                    cbn = min(cb, nb - j0)
                    blks = slice(nb_off + j0, nb_off + j0 + cbn)
                    sa = sap.tile([P, cb, D], BF16, tag="sa")
                    nc.sync.dma_start(
                        out=sa[:, :cbn],
                        in_=SA[:, a_off + j0 * D : a_off + (j0 + cbn) * D]
                        .rearrange("p (c d) -> p c d", d=D))
                    if layer == 1:
                        # w lives as feature row Fh of q: z rides the q tree
                        q = shp.tile([P, cb, FT, D], BF16, tag="q")
                        nc.sync.dma_start(
                            out=q[:, :cbn, 0:Fh, :],
                            in_=SH[:, h_off + j0 * Fh * D : h_off + (j0 + cbn) * Fh * D]
                            .rearrange("p (c k d) -> p c k d", k=Fh, d=D))
                        wv = q[:, :cbn, Fh:FT, :]
                    else:
                        # separate w tile keeps the SH DMA fully contiguous
                        q = shp.tile([P, cb, Fh, D], BF16, tag="q")
                        nc.sync.dma_start(
                            out=q[:, :cbn],
                            in_=SH[:, h_off + j0 * Fh * D : h_off + (j0 + cbn) * Fh * D]
                            .rearrange("p (c k d) -> p c k d", k=Fh, d=D))
                        w = wpp.tile([P, cb, 1, D], BF16, tag="w")
                        wv = w[:, :cbn]
                    # e = s_src + s_dst (Pool); w = exp(leaky_relu(e, .2)) (ACT)
                    nc.gpsimd.tensor_tensor(
                        out=sa[:, :cbn, :], in0=sa[:, :cbn, :],
                        in1=tb[:, blks, :].to_broadcast([P, cbn, D]),
                        op=ALU.add)
                    nc.scalar.activation(sa[:, :cbn, :], sa[:, :cbn, :],
                                         AF.Prelu, alpha=0.2)
                    nc.scalar.activation(wv[:, :, 0, :], sa[:, :cbn, :],
                                         AF.Exp)

                    if layer == 1:
                        # mults: DVE f0:7, Pool f7; trees: DVE f0:7, Pool f7:9
                        nc.vector.tensor_tensor(
                            out=q[:, :cbn, 0:7, :], in0=q[:, :cbn, 0:7, :],
                            in1=wv.to_broadcast([P, cbn, 7, D]), op=ALU.mult)
                        nc.gpsimd.tensor_tensor(
                            out=q[:, :cbn, 7:8, :], in0=q[:, :cbn, 7:8, :],
                            in1=wv.to_broadcast([P, cbn, 1, D]), op=ALU.mult)
                        d = tree4(nc.vector, q[:, :cbn], 0, 7, D)
                        tree4(nc.gpsimd, q[:, :cbn], 7, FT, D)
                        nc.vector.tensor_reduce(
                            out=gacc[:, blks, :],
                            in_=q[:, :cbn, :, 0:d], axis=AX.X, op=ALU.add)
                    else:
                        # mults/feature-trees on DVE; w(z)-tree on Pool
                        nc.vector.tensor_tensor(
                            out=q[:, :cbn, :, :], in0=q[:, :cbn, :, :],
                            in1=wv.to_broadcast([P, cbn, Fh, D]), op=ALU.mult)
                        d = tree4(nc.vector, q[:, :cbn], 0, Fh, D)
                        dw = tree4(nc.vector, w[:, :cbn], 0, 1, D)
                        nc.vector.tensor_reduce(
                            out=gacc[:, blks, 0:Fh],
                            in_=q[:, :cbn, :, 0:d], axis=AX.X, op=ALU.add)
                        nc.vector.tensor_reduce(
                            out=gacc[:, blks, Fh:FT],
                            in_=w[:, :cbn, :, 0:dw], axis=AX.X, op=ALU.add)
                h_off += nb * Fh * D
                a_off += nb * D
                nb_off += nb

            emit_tail(tail_upto[0], n_pb)
    nc.finalize()
    return nc


# ----------------------------------------------------------------------
# Orchestration
# ----------------------------------------------------------------------

def run_gat(x, W1, a1_src, a1_dst, b1, W2, a2_src, a2_dst, b2, edge_index):
    N = x.shape[0]
    src = np.asarray(edge_index[0], np.int64)
    dst = np.asarray(edge_index[1], np.int64)
    plan = build_plan(dst, N)
    n_pb = plan["n_pb"]
    cores = list(range(N_CORES))

    # ---------------- L0 ----------------
    nc0 = build_l0()
    xpad = np.zeros((N_CORES, NC_PAD, 36), bf16)
    xpad[:, :NC_NODES] = np.asarray(x).reshape(N_CORES, NC_NODES, 36).astype(bf16)
    W1f = np.asarray(W1, np.float32)
    AVf = np.stack([np.asarray(a1_src, np.float32), np.asarray(a1_dst, np.float32)], 1)
    in_maps0 = []
    W1TA = np.ascontiguousarray(np.concatenate([W1f.T, AVf], axis=1))
    for c in cores:
        in_maps0.append({
            "xT": np.ascontiguousarray(xpad[c].T),
            "W1": W1f, "W1TA": W1TA,
        })
    r0 = run_bass_kernel_spmd(nc0, in_maps0, cores).results
    h1bf = np.zeros((N, 8), bf16)
    s1s = np.zeros((N,), np.float32)
    s1d = np.zeros((N,), np.float32)
    for c in cores:
        # node t*P+p lives at [p, t, :]
        hcore = np.asarray(r0[c]["HOUT"]).reshape(P, NT0, 8).transpose(1, 0, 2)
        score = np.asarray(r0[c]["SOUT"]).reshape(P, NT0, 2).transpose(1, 0, 2)
        sl = slice(c * NC_NODES, (c + 1) * NC_NODES)
        h1bf[sl] = hcore.reshape(NC_PAD, 8)[:NC_NODES]
        s = score.reshape(NC_PAD, 2)[:NC_NODES]
        s1s[sl] = s[:, 0]
        s1d[sl] = s[:, 1]

    # ---------------- L1 ----------------
    nc1 = build_reduce_layer(plan, 8, 1)
    b1rep = np.tile(np.asarray(b1, np.float32)[None, :], (P, 1))
    W2rep = np.tile(np.asarray(W2, np.float32).T.reshape(1, 2, 8), (P, 1, 1)).reshape(P, 16)
    A2Vrep = np.tile(np.stack([np.asarray(a2_src, np.float32), np.asarray(a2_dst, np.float32)], 0).reshape(1, 2, 2), (P, 1, 1)).reshape(P, 4)
    in_maps1 = []
    for c in cores:
        SAc, SHc = build_streams(plan, c, src, s1s, h1bf)
        TB = build_node_tab(plan, c, s1d[:, None], 1).reshape(P, -1)
        in_maps1.append({
            "SA": SAc, "SH": SHc, "TB": TB, "BIAS": b1rep,
            "W2R": W2rep, "A2V": A2Vrep,
        })
    r1 = run_bass_kernel_spmd(nc1, in_maps1, cores).results

    # host: rebuild global tables for L2
    h2bf = np.zeros((N, 2), bf16)
    s2s = np.zeros((N,), np.float32)
    s2d = np.zeros((N,), np.float32)
    for c in cores:
        no = plan["cores"][c]["node_order"].reshape(n_pb, P)
        valid = no >= 0
        outh = np.asarray(r1[c]["TOUTH"]).reshape(P, n_pb, 2).transpose(1, 0, 2)
        outs = np.asarray(r1[c]["TOUTS"]).reshape(P, n_pb, 2).transpose(1, 0, 2)
        h2bf[no[valid]] = outh[valid]
        s2s[no[valid]] = outs[valid][:, 0]
        s2d[no[valid]] = outs[valid][:, 1]

    # ---------------- L2 ----------------
    nc2 = build_reduce_layer(plan, 2, 2)
    b2rep = np.tile(np.asarray(b2, np.float32)[None, :], (P, 1))
    in_maps2 = []
    for c in cores:
        SAc, SHc = build_streams(plan, c, src, s2s, h2bf)
        TB = build_node_tab(plan, c, s2d[:, None], 1).reshape(P, -1)
        in_maps2.append({"SA": SAc, "SH": SHc, "TB": TB, "BIAS": b2rep})
    r2 = run_bass_kernel_spmd(nc2, in_maps2, cores).results

    out = np.zeros((N, 2), np.float32)
    for c in cores:
        no = plan["cores"][c]["node_order"].reshape(n_pb, P)
        valid = no >= 0
        o = np.asarray(r2[c]["TOUT2"]).reshape(P, n_pb, 2).transpose(1, 0, 2)
        out[no[valid]] = o[valid]
    return out


def kernel(**inputs):
    out = run_gat(
        x=np.asarray(inputs["x"], np.float32),
        W1=np.asarray(inputs["W1"], np.float32),
        a1_src=np.asarray(inputs["a1_src"], np.float32),
        a1_dst=np.asarray(inputs["a1_dst"], np.float32),
        b1=np.asarray(inputs["b1"], np.float32),
        W2=np.asarray(inputs["W2"], np.float32),
        a2_src=np.asarray(inputs["a2_src"], np.float32),
        a2_dst=np.asarray(inputs["a2_dst"], np.float32),
        b2=np.asarray(inputs["b2"], np.float32),
        edge_index=np.asarray(inputs["edge_index"]),
    )
    return out.astype(np.float32)
